# revision 38
# baseline (speedup 1.0000x reference)
"""Trainium2 Bass kernel for nn_EncoderDecoderTransformer (sparse kNN encoder attention).

Sharding: data-parallel over batch (4 batches x 2 cores); each pair of cores
splits the sequence (512 tokens each). Per attention sub-layer the pair
AllGathers the LN output h (bf16, 512KB) right after layer norm; each core then
projects K/V for the FULL sequence locally (PE has headroom), so the collective
is off the critical path (Q/K/V-own projections overlap the flight).
Cross-attention K/V are projected per decoder layer from a one-time AllGather
of enc_out - no per-layer cross collectives.

Layouts (per core):
  - Activations feature-major: x^T stored as 4 tiles (128 dims, 512 own tokens).
  - Q^T feature-major (head h lives in rows [64*(h%2):...] of ptile h//2).
  - K^T feature-major full-seq: 4 tiles [128, 1024] (columns = absolute token).
  - V token-major full-seq: 8 tiles (128 tokens, 8 heads, 65) with a constant-1
    column per head so the AV matmul also produces the softmax denominator.
  - Scores transposed: S^T = K^T.T @ Q^T, with BOTH heads of a ptile batched
    into one [128, 1024] PSUM tile so the Exp activation runs at N=1024
    (amortizes the ~352-cycle ACT overhead). kNN/causal masking is a 0/1 bf16
    multiply with column-duplicated [128,1024] mask tiles.
  - Decoder self-attention skips fully-masked key tiles (half-0 cores do 4 of
    8 kt) and skips the mask multiply on fully-allowed tiles (half-1, kt 0-3).
  - kNN mask: s'_qk = 2 x_q.x_k - |x_k|^2 orders like -distance; rank-17
    threshold via DVE max8/match_replace. Same math as the verified baseline.
"""

import os
import numpy as np
import ml_dtypes

BF16 = ml_dtypes.bfloat16

D, F, H, NE, ND, KNN = 512, 2048, 8, 4, 4, 16
B, LE, LD = 4, 1024, 1024
DH = D // H
NCORE = 8
P = 128
TOWN = 512          # tokens owned per core
T2 = 2 * TOWN       # batched free dim (two heads side by side)
NDT = D // P        # 4 feature tiles
NKT = LE // P       # 8 key tiles
NEG = -1e30
EPS = 1e-5
PAIRS = [[0, 1], [2, 3], [4, 5], [6, 7]]

_CACHE = {}


def build(n_enc=NE, n_dec=ND):
    from contextlib import ExitStack

    import concourse.bacc as bacc
    import concourse.tile as tile
    import concourse.mybir as mybir

    f32 = mybir.dt.float32
    bf16 = mybir.dt.bfloat16
    AF = mybir.ActivationFunctionType
    OP = mybir.AluOpType

    nc = bacc.Bacc("TRN2", target_bir_lowering=False, debug=False, num_devices=NCORE)

    # ---- I/O ----
    def din(name, shape, dt=f32):
        return nc.dram_tensor(name, shape, dt, kind="ExternalInput")

    x0T = din("x0T", [NDT, P, TOWN])
    y0T = din("y0T", [NDT, P, TOWN])
    xq2_d = din("xq2", [TOWN, 3])       # 2*xyz for own tokens
    xrow_d = din("xrow", [4, LE])        # [xyz, |xyz|^2] all tokens, transposed
    boscol_d = din("boscol", [TOWN, 1])  # threshold override col (NEG at BOS q)
    eye_in = din("eye", [P, P])
    causal_in = din("causal", [NKT, P, TOWN], bf16)  # per-core causal kt tiles

    ew_qkv = din("ew_qkv", [NE, D, 3 * D], bf16)
    ew_out = din("ew_out", [NE, D, D], bf16)
    ew_f1 = din("ew_f1", [NE, D, F], bf16)
    ew_f2 = din("ew_f2", [NE, F, D], bf16)
    eb_qkv = din("eb_qkv", [NE, 3 * D, 1])
    eb_out = din("eb_out", [NE, D, 1])
    eb_f1 = din("eb_f1", [NE, F, 1])
    eb_f2 = din("eb_f2", [NE, D, 1])

    dw_saqkv = din("dw_saqkv", [ND, D, 3 * D], bf16)
    db_saqkv = din("db_saqkv", [ND, 3 * D, 1])
    dw_saout = din("dw_saout", [ND, D, D], bf16)
    db_saout = din("db_saout", [ND, D, 1])
    dw_caqkv = din("dw_caqkv", [ND, D, 3 * D], bf16)
    db_caqkv = din("db_caqkv", [ND, 3 * D, 1])
    dw_caout = din("dw_caout", [ND, D, D], bf16)
    db_caout = din("db_caout", [ND, D, 1])
    dw_f1 = din("dw_f1", [ND, D, F], bf16)
    db_f1 = din("db_f1", [ND, F, 1])
    dw_f2 = din("dw_f2", [ND, F, D], bf16)
    db_f2 = din("db_f2", [ND, D, 1])
    eb_qkv_bf = din("eb_qkv_bf", [NE, 3 * D, 1], bf16)
    db_saqkv_bf = din("db_saqkv_bf", [ND, 3 * D, 1], bf16)
    db_caqkv_bf = din("db_caqkv_bf", [ND, 3 * D, 1], bf16)

    enc_part = nc.dram_tensor("enc_part", [NDT, P, TOWN], f32, kind="ExternalOutput")
    dec_part = nc.dram_tensor("dec_part", [NDT, P, TOWN], f32, kind="ExternalOutput")

    with tile.TileContext(nc) as tc, ExitStack() as ctx:
        ep = ctx.enter_context

        pc = ep(tc.tile_pool(name="pc", bufs=1))
        p_allow = ep(tc.tile_pool(name="p_allow", bufs=8))
        p_causal = ep(tc.tile_pool(name="p_causal", bufs=8))
        ps_s = ep(tc.tile_pool(name="ps_s", bufs=2, space="PSUM"))
        ps_o = ep(tc.tile_pool(name="ps_o", bufs=2, space="PSUM"))
        ps_mm = ep(tc.tile_pool(name="ps_mm", bufs=2, space="PSUM"))
        p_dram = ep(tc.tile_pool(name="p_dram", bufs=2, space="DRAM"))

        # ---- constants ----
        ones_col_bf = pc.tile([P, 1], bf16)
        nc.vector.memset(ones_col_bf, 1.0)
        ones_row = pc.tile([1, P], f32)
        nc.vector.memset(ones_row, 1.0)
        ones_row_bf = pc.tile([1, P], bf16)
        nc.vector.memset(ones_row_bf, 1.0)
        eps_sb = pc.tile([1, 1], f32)
        nc.vector.memset(eps_sb, EPS)
        dummy_w = pc.tile([P, P], bf16)
        nc.vector.memset(dummy_w, 0.0)
        dummy_x = pc.tile([P, TOWN], bf16)
        nc.vector.memset(dummy_x, 0.0)

        eye_f32 = pc.tile([P, P], f32)
        nc.sync.dma_start(out=eye_f32, in_=eye_in[:, :])

        causal_sb = []
        for i in range(NKT):
            t = p_causal.tile([P, TOWN], bf16, tag="causal")
            nc.sync.dma_start(out=t, in_=causal_in[i])
            causal_sb.append(t)

        def pe_warm(n):
            psD = ps_mm.tile([P, TOWN], f32, tag="mm")
            for _ in range(n):
                nc.tensor.matmul(psD, dummy_w, dummy_x, start=True, stop=True)

        def build_mask():
            # Exact-fp32 kNN scores s3 = 2 x_q.x_k - |x_k|^2 (orders like
            # -distance), computed query-major once; the rank-17 value (16 NN +
            # self) is the inclusion threshold. allow = (s3 >= thr) compares the
            # SAME fp32 values the threshold came from, then the 0/1 bf16 mask
            # is moved to key-major via exact PE transposes.
            import concourse.bass as cbass

            def bcast_rows(dram_row_ap, pool, n_free, tag):
                t = pool.tile([P, n_free], f32, tag=tag)
                src_ap = cbass.AP(
                    tensor=dram_row_ap.tensor, offset=dram_row_ap.offset,
                    ap=[[0, P]] + list(dram_row_ap.ap),
                )
                nc.sync.dma_start(out=t, in_=src_ap)
                return t

            allow_sb = []
            for kt in range(NKT):
                t = p_allow.tile([P, T2], bf16, tag="allow", name=f"allow{kt}")
                allow_sb.append(t)
            with tc.tile_pool(name="p_mask", bufs=3) as p_mask, \
                 tc.tile_pool(name="p_mbc", bufs=1) as p_mbc, \
                 tc.tile_pool(name="p_m8", bufs=8) as p_m8, \
                 tc.tile_pool(name="p_alq", bufs=2) as p_alq:
                bcx = []
                for c in range(4):
                    t = bcast_rows(xrow_d[c], p_mbc, LE, tag=f"bcx{c}")
                    bcx.append(t)
                for qt in range(4):
                    xqc = p_m8.tile([P, 3], f32, tag="xqc")
                    nc.sync.dma_start(out=xqc, in_=xq2_d[qt * P:(qt + 1) * P, :])
                    bosc = p_m8.tile([P, 1], f32, tag="bosc")
                    nc.sync.dma_start(out=bosc, in_=boscol_d[qt * P:(qt + 1) * P, :])
                    s0 = p_mask.tile([P, LE], f32, tag="s")
                    nc.vector.tensor_scalar(s0, bcx[0], xqc[:, 0:1], None, op0=OP.mult)
                    s1 = p_mask.tile([P, LE], f32, tag="s")
                    nc.vector.scalar_tensor_tensor(s1, bcx[1], xqc[:, 1:2], s0, OP.mult, OP.add)
                    s2 = p_mask.tile([P, LE], f32, tag="s")
                    nc.vector.scalar_tensor_tensor(s2, bcx[2], xqc[:, 2:3], s1, OP.mult, OP.add)
                    s3 = p_mask.tile([P, LE], f32, tag="s")
                    nc.vector.tensor_tensor(s3, s2, bcx[3], OP.subtract)
                    psW = ps_s.tile([P, T2], f32, tag="pss")
                    nc.tensor.matmul(psW[:, 0:TOWN], s3[:, 0:P], bcx[0][:, 0:TOWN],
                                     start=True, stop=True)
                    m8 = p_m8.tile([P, 8], f32, tag="m8")
                    nc.vector.max(m8, s3)
                    s4 = p_mask.tile([P, LE], f32, tag="s")
                    nc.vector.match_replace(s4, m8, s3, NEG)
                    m8b = p_m8.tile([P, 8], f32, tag="m8")
                    nc.vector.max(m8b, s4)
                    s5 = p_mask.tile([P, LE], f32, tag="s")
                    nc.vector.match_replace(s5, m8b, s4, NEG)
                    nc.tensor.matmul(psW[:, TOWN:T2], s5[:, 0:P], bcx[0][:, 0:TOWN],
                                     start=True, stop=True)
                    m8c = p_m8.tile([P, 8], f32, tag="m8")
                    nc.vector.max(m8c, s5)
                    thr = p_m8.tile([P, 1], f32, tag="thr")
                    nc.vector.tensor_tensor(thr, m8c[:, 0:1], bosc, OP.min)
                    al_q = p_alq.tile([P, LE], f32, tag="alq")
                    nc.vector.tensor_scalar(al_q, s3, thr, None, op0=OP.is_ge)
                    for kt in range(NKT):
                        psT = ps_mm.tile([P, P], f32, tag="mm")
                        nc.tensor.transpose(psT, al_q[:, kt * P:(kt + 1) * P], eye_f32)
                        nc.vector.tensor_copy(
                            allow_sb[kt][:, qt * P:(qt + 1) * P], psT
                        )
                for kt in range(NKT):
                    nc.vector.tensor_copy(allow_sb[kt][:, TOWN:T2], allow_sb[kt][:, 0:TOWN])
                nc.vector.memset(allow_sb[0][0:1, :], 1.0)  # BOS key allowed for all q
            return allow_sb

        pe_warm(12)
        allow_sb = build_mask()

        # ================= helpers =================
        def load_w(pool, dram_ap, kchunks, cols, tag):
            t = pool.tile([P, kchunks, cols], bf16, tag=tag)
            nc.sync.dma_start(
                out=t, in_=dram_ap.rearrange("(kc p) m -> p kc m", p=P)
            )
            return t

        def ps_alt(i):
            if i % 2 == 0:
                return ps_mm.tile([P, TOWN], f32, tag="mm", name=f"psa{i}")
            return ps_s.tile([P, TOWN], f32, tag="pss", name=f"psb{i}")

        def layer_norm(xs, out_dt, out_pool, out_tag):
            sqs = []
            xbs = []
            for dt in range(NDT):
                sq = p_lnsq.tile([P, TOWN], bf16, tag="lnsq")
                nc.scalar.activation(sq, xs[dt], AF.Square)
                sqs.append(sq)
                xb = p_lnsq.tile([P, TOWN], bf16, tag="lnxb")
                nc.vector.tensor_copy(xb, xs[dt])
                xbs.append(xb)
            ps_mean = ps_mm.tile([1, TOWN], f32, tag="mm")
            for dt in range(NDT):
                nc.tensor.matmul(ps_mean, ones_col_bf, xbs[dt], start=dt == 0, stop=dt == 3)
            ps_sq = ps_mm.tile([1, TOWN], f32, tag="mm")
            for dt in range(NDT):
                nc.tensor.matmul(ps_sq, ones_col_bf, sqs[dt], start=dt == 0, stop=dt == 3)
            psW = ps_s.tile([P, T2], f32, tag="pss")
            mu = p_small.tile([1, TOWN], f32, tag="sm")
            nc.vector.tensor_single_scalar(mu, ps_mean, 1.0 / D, OP.mult)
            musq = p_small.tile([1, TOWN], f32, tag="sm")
            nc.vector.tensor_tensor(musq, mu, mu, OP.mult)
            nc.tensor.matmul(psW[:, 0:TOWN], ones_row, mu, start=True, stop=True)
            var = p_small.tile([1, TOWN], f32, tag="sm")
            nc.vector.scalar_tensor_tensor(var, ps_sq, 1.0 / D, musq, OP.mult, OP.subtract)
            nc.tensor.matmul(psW[:, 0:TOWN], ones_row, var, start=True, stop=True)
            lnv = p_small.tile([1, TOWN], f32, tag="sm")
            nc.scalar.activation(lnv, var, AF.Ln, bias=eps_sb)
            nc.tensor.matmul(psW[:, 0:TOWN], ones_row, lnv, start=True, stop=True)
            rstd = p_small.tile([1, TOWN], f32, tag="sm")
            nc.scalar.activation(rstd, lnv, AF.Exp, scale=-0.5)
            nc.tensor.matmul(psW[:, 0:TOWN], ones_row, rstd, start=True, stop=True)
            cro = p_small.tile([1, TOWN], f32, tag="sm")
            nc.vector.scalar_tensor_tensor(cro, mu, -1.0, rstd, OP.mult, OP.mult)
            ps_a = ps_mm.tile([P, TOWN], f32, tag="mm")
            nc.tensor.matmul(ps_a, ones_row, rstd, start=True, stop=True)
            ps_c = ps_mm.tile([P, TOWN], f32, tag="mm")
            nc.tensor.matmul(ps_c, ones_row, cro, start=True, stop=True)
            a_sb = p_lnac.tile([P, TOWN], f32, tag="lna")
            nc.vector.tensor_copy(a_sb, ps_a)
            c_sb = p_lnac.tile([P, TOWN], f32, tag="lnc")
            nc.vector.tensor_copy(c_sb, ps_c)
            hs = []
            for dt in range(NDT):
                h = out_pool.tile([P, TOWN], out_dt, tag=out_tag)
                nc.vector.tensor_tensor(h, xs[dt], a_sb, OP.mult)
                nc.vector.tensor_tensor(h, h, c_sb, OP.add)
                hs.append(h)
            return hs

        def proj_fm(w_sb, col_off, n_m, rhs, bias_ap, out_pool, out_tag, out_dt=bf16):
            """Feature-major projection; per-partition bias applied on eviction."""
            outs = []
            nk = len(rhs)
            for m in range(n_m):
                ps = ps_alt(m)
                for kc in range(nk):
                    nc.tensor.matmul(
                        ps, w_sb[:, kc, col_off + m * P:col_off + (m + 1) * P],
                        rhs[kc], start=kc == 0, stop=kc == nk - 1,
                    )
                bcol = p_bias.tile([P, 1], f32, tag="bcol")
                nc.sync.dma_start(out=bcol, in_=bias_ap[col_off + m * P:col_off + (m + 1) * P, :])
                o = out_pool.tile([P, TOWN], out_dt, tag=out_tag)
                nc.vector.tensor_scalar(o, ps, bcol, None, op0=OP.add)
                outs.append(o)
            return outs

        def kv_project(wk_sb, wv_sb, hs_half, hh, bias_ap, bias_bf_ap, Ks, Vs):
            """Project K (feature-major, into Ks column half hh) and V (token-
            major tiles Vs[4*hh + tt]) for the 512 tokens of half hh.
            bias_ap/bias_bf_ap hold [3D] packed qkv bias; K at +D, V at +2D."""
            csl = slice(hh * TOWN, (hh + 1) * TOWN)
            for m in range(NDT):
                ps = ps_alt(m)
                for kc in range(NDT):
                    nc.tensor.matmul(
                        ps, wk_sb[:, kc, m * P:(m + 1) * P],
                        hs_half[kc], start=kc == 0, stop=kc == NDT - 1,
                    )
                bcol = p_bias.tile([P, 1], f32, tag="bcol")
                nc.sync.dma_start(out=bcol, in_=bias_ap[D + m * P:D + (m + 1) * P, :])
                nc.scalar.activation(Ks[m][:, csl], ps, AF.Identity, bias=bcol)
            brow512 = p_bias.tile([1, TOWN], bf16, tag="brow512")
            nc.sync.dma_start(
                out=brow512,
                in_=bias_bf_ap[2 * D:3 * D, :].rearrange("a b -> b a"),
            )
            for tt in range(4):
                ps = ps_alt(tt)
                for kc in range(NDT):
                    nc.tensor.matmul(
                        ps, hs_half[kc][:, tt * P:(tt + 1) * P],
                        wv_sb[:, kc, 0:D],
                        start=kc == 0, stop=False,
                    )
                nc.tensor.matmul(ps, ones_row_bf, brow512, start=False, stop=True)
                vt = Vs[4 * hh + tt]
                nc.vector.tensor_copy(
                    vt[:, :, 0:64],
                    ps.rearrange("p (h d) -> p h d", h=H),
                )

        def alloc_vs():
            Vs = []
            for tt in range(NKT):
                vt = p_v.tile([P, H, 65], bf16, tag="vsb")
                nc.vector.memset(vt[:, :, 64:65], 1.0)
                Vs.append(vt)
            return Vs

        def attention(Qs, Ks, Vs, kt_order, masks, uniq):
            """masks: dict kt -> ("dup", [P,T2] tile) | ("half", [P,TOWN] tile).
            S matmuls run two kt ahead of AV so the PE FIFO never stalls on the
            exp/mask chain; softmax 1/den is broadcast via DMA (off the PE)."""
            OTs = []
            nkt = len(kt_order)
            for hp in range(4):
                psO = []
                for _j in range(2):
                    psO_t = ps_o.tile([65, TOWN], f32, tag="pso")
                    psO.append(psO_t)
                e2s = {}

                def emit_S(idx, hp=hp, e2s=e2s):
                    kt = kt_order[idx]
                    psS = ps_s.tile([P, T2], f32, tag="pss")
                    for j in range(2):
                        nc.tensor.matmul(
                            psS[:, j * TOWN:(j + 1) * TOWN],
                            Ks[hp][j * 64:(j + 1) * 64, kt * P:(kt + 1) * P],
                            Qs[hp][j * 64:(j + 1) * 64, :],
                            start=True, stop=True,
                        )
                    e = p_e.tile([P, T2], bf16, tag="e")
                    nc.scalar.activation(e, psS, AF.Exp, scale=0.125)
                    m = masks.get(kt)
                    if m is not None:
                        kind, mt = m
                        e2 = p_e.tile([P, T2], bf16, tag="e")
                        if kind == "dup":
                            nc.vector.tensor_tensor(e2, e, mt, OP.mult)
                        else:
                            for j in range(2):
                                jsl = slice(j * TOWN, (j + 1) * TOWN)
                                nc.vector.tensor_tensor(e2[:, jsl], e[:, jsl], mt, OP.mult)
                    else:
                        e2 = e
                    e2s[idx] = e2

                def emit_AV(idx, hp=hp, e2s=e2s, psO=psO):
                    kt = kt_order[idx]
                    e2 = e2s.pop(idx)
                    for j in range(2):
                        nc.tensor.matmul(
                            psO[j], Vs[kt][:, 2 * hp + j, :],
                            e2[:, j * TOWN:(j + 1) * TOWN],
                            start=idx == 0, stop=idx == nkt - 1,
                        )

                emit_S(0)
                if nkt > 1:
                    emit_S(1)
                for idx in range(nkt):
                    emit_AV(idx)
                    if idx + 2 < nkt:
                        emit_S(idx + 2)
                ot = p_ot.tile([P, TOWN], bf16, tag="ot")
                for j in range(2):
                    den = p_small.tile([1, TOWN], f32, tag="sm")
                    nc.vector.tensor_copy(den, psO[j][64:65, :])
                    rec = p_small.tile([1, TOWN], f32, tag="sm")
                    nc.vector.reciprocal_approx_fast(rec, den)
                    recd = p_dram.tile([1, TOWN], f32, tag="recb")
                    nc.sync.dma_start(out=recd, in_=rec)
                    bc = p_bc.tile([64, TOWN], f32, tag="bc")
                    import concourse.bass as cbass
                    src_ap = cbass.AP(
                        tensor=recd[0].tensor, offset=recd[0].offset,
                        ap=[[0, 64]] + list(recd[0].ap),
                    )
                    nc.sync.dma_start(out=bc, in_=src_ap)
                    nc.vector.tensor_tensor(ot[j * 64:(j + 1) * 64, :], psO[j][0:64, :], bc, OP.mult)
                OTs.append(ot)
            return OTs

        def proj_residual(w_sb, col_off, n_k, rhs, bias_ap, xs):
            nxs = []
            for m in range(NDT):
                ps = ps_alt(m)
                for kc in range(n_k):
                    nc.tensor.matmul(
                        ps, w_sb[:, kc, col_off + m * P:col_off + (m + 1) * P],
                        rhs[kc], start=kc == 0, stop=kc == n_k - 1,
                    )
                bcol = p_bias.tile([P, 1], f32, tag="bcol")
                nc.sync.dma_start(out=bcol, in_=bias_ap[m * P:(m + 1) * P, :])
                nx = p_x.tile([P, TOWN], f32, tag="x")
                nc.vector.scalar_tensor_tensor(nx, ps, bcol, xs[m], OP.add, OP.add)
                nxs.append(nx)
            return nxs

        def ffn(w1_ap, w2_ap, b1_ap, b2_ap, hs, xs):
            gs = []
            for m in range(F // P):
                w1m = p_w1.tile([P, NDT, P], bf16, tag="wf1")
                nc.sync.dma_start(
                    out=w1m,
                    in_=w1_ap[:, m * P:(m + 1) * P].rearrange("(kc p) m -> p kc m", p=P),
                )
                ps = ps_alt(m)
                for kc in range(NDT):
                    nc.tensor.matmul(
                        ps, w1m[:, kc, :], hs[kc],
                        start=kc == 0, stop=kc == NDT - 1,
                    )
                bcol = p_bias.tile([P, 1], f32, tag="bcol")
                nc.sync.dma_start(out=bcol, in_=b1_ap[m * P:(m + 1) * P, :])
                g = p_g.tile([P, TOWN], bf16, tag="g")
                nc.scalar.activation(g, ps, AF.Gelu, bias=bcol)
                gs.append(g)
            nxs = []
            for m in range(NDT):
                w2m = p_w2.tile([P, F // P, P], bf16, tag="wf2")
                nc.sync.dma_start(
                    out=w2m,
                    in_=w2_ap[:, m * P:(m + 1) * P].rearrange("(kc p) c -> p kc c", p=P),
                )
                ps2 = ps_alt(m)
                for kc in range(F // P):
                    nc.tensor.matmul(
                        ps2, w2m[:, kc, :], gs[kc],
                        start=kc == 0, stop=kc == F // P - 1,
                    )
                bcol = p_bias.tile([P, 1], f32, tag="bcol")
                nc.sync.dma_start(out=bcol, in_=b2_ap[m * P:(m + 1) * P, :])
                nx = p_x.tile([P, TOWN], f32, tag="x")
                nc.vector.scalar_tensor_tensor(nx, ps2, bcol, xs[m], OP.add, OP.add)
                nxs.append(nx)
            return nxs

        def ag_h(hs, uniq):
            """DMA h tiles to a DRAM bounce and AllGather across the pair.
            Returns the gathered [2, NDT, P, TOWN] DRAM tile (index = half)."""
            hbin = p_dram.tile([NDT, P, TOWN], bf16, tag=f"hbin{uniq}")
            for dt in range(NDT):
                nc.sync.dma_start(out=hbin[dt], in_=hs[dt])
            hbout = p_dram.tile([2, NDT, P, TOWN], bf16, tag=f"hbout{uniq}")
            nc.gpsimd.collective_compute(
                "AllGather", OP.bypass, replica_groups=PAIRS,
                ins=[hbin[:].opt()], outs=[hbout[:].opt()],
            )
            return hbout

        def load_h_halves(hbout):
            halves = []
            for hh in range(2):
                tiles = []
                for dt in range(NDT):
                    t = p_h.tile([P, TOWN], bf16, tag="hall")
                    nc.sync.dma_start(out=t, in_=hbout[hh, dt])
                    tiles.append(t)
                halves.append(tiles)
            return halves

        p_x = ep(tc.tile_pool(name="p_x", bufs=5))
        p_h = ep(tc.tile_pool(name="p_h", bufs=12))
        p_q = ep(tc.tile_pool(name="p_q", bufs=5))
        p_kv = ep(tc.tile_pool(name="p_kv", bufs=8))
        p_v = ep(tc.tile_pool(name="p_v", bufs=9))
        p_ot = ep(tc.tile_pool(name="p_ot", bufs=4))
        p_e = ep(tc.tile_pool(name="p_e", bufs=4))
        p_g = ep(tc.tile_pool(name="p_g", bufs=16))
        p_lnsq = ep(tc.tile_pool(name="p_lnsq", bufs=4))
        p_lnac = ep(tc.tile_pool(name="p_lnac", bufs=2))
        p_bc = ep(tc.tile_pool(name="p_bc", bufs=2))
        p_small = ep(tc.tile_pool(name="p_small", bufs=6))
        p_bias = ep(tc.tile_pool(name="p_bias", bufs=4))
        p_eo = ep(tc.tile_pool(name="p_eo", bufs=2))
        p_eoball = ep(tc.tile_pool(name="p_eoball", bufs=4))
        p_w1 = ep(tc.tile_pool(name="p_w1", bufs=2))
        p_w2 = ep(tc.tile_pool(name="p_w2", bufs=2))
        p_w = ep(tc.tile_pool(name="p_w", bufs=3))

        all_kt = list(range(NKT))
        sa_masks = {kt: ("half", causal_sb[kt]) for kt in range(NKT)}
        enc_masks = {kt: ("dup", allow_sb[kt]) for kt in range(NKT)}

        # ================= encoder =================
        xs = []
        for dt in range(NDT):
            x = p_x.tile([P, TOWN], f32, tag="x")
            nc.sync.dma_start(out=x, in_=x0T[dt])
            xs.append(x)

        for l in range(n_enc):
            wq = load_w(p_w, ew_qkv[l][:, 0:D], NDT, D, "w")
            wk = load_w(p_w, ew_qkv[l][:, D:2 * D], NDT, D, "w")
            wv = load_w(p_w, ew_qkv[l][:, 2 * D:3 * D], NDT, D, "w")
            wout = load_w(p_w, ew_out[l], NDT, D, "w")

            hs = layer_norm(xs, bf16, p_h, "h")
            hbout = ag_h(hs, f"e{l}")
            Qs = proj_fm(wq, 0, 4, hs, eb_qkv[l], p_q, "q")
            Ks = [p_kv.tile([P, LE], bf16, tag="ksb", name=f"ks{m}") for m in range(NDT)]
            Vs = alloc_vs()
            hhalves = load_h_halves(hbout)
            for hh in range(2):
                kv_project(wk, wv, hhalves[hh], hh, eb_qkv[l], eb_qkv_bf[l], Ks, Vs)
            OTs = attention(Qs, Ks, Vs, all_kt, enc_masks, f"e{l}")
            xs = proj_residual(wout, 0, NDT, OTs, eb_out[l], xs)
            hs = layer_norm(xs, bf16, p_h, "h")
            xs = ffn(ew_f1[l], ew_f2[l], eb_f1[l], eb_f2[l], hs, xs)

        eof = layer_norm(xs, f32, p_eo, "eof")
        eob = []
        for dt in range(NDT):
            nc.sync.dma_start(out=enc_part[dt], in_=eof[dt])
            t = p_h.tile([P, TOWN], bf16, tag="eob")
            nc.vector.tensor_copy(t, eof[dt])
            eob.append(t)
        ebout = ag_h(eob, "eo")
        eob_all = []
        for dt in range(NDT):
            t = p_eoball.tile([P, LE], bf16, tag="eoball")
            for hh in range(2):
                nc.sync.dma_start(
                    out=t[:, hh * TOWN:(hh + 1) * TOWN], in_=ebout[hh, dt]
                )
            eob_all.append(t)

        # ================= decoder =================
        ys = []
        for dt in range(NDT):
            y = p_x.tile([P, TOWN], f32, tag="x")
            nc.sync.dma_start(out=y, in_=y0T[dt])
            ys.append(y)

        def ca_k_project(l, wkv):
            caK = [p_kv.tile([P, LE], bf16, tag="ksb", name=f"cak{m}") for m in range(NDT)]
            for m in range(NDT):
                ps = ps_mm.tile([P, TOWN], f32, tag="mm")
                for kc in range(NDT):
                    nc.tensor.matmul(
                        ps, wkv[:, kc, m * P:(m + 1) * P],
                        eob_all[kc][:, 0:TOWN], start=kc == 0, stop=kc == NDT - 1,
                    )
                ps2 = ps_s.tile([P, TOWN], f32, tag="pss")
                for kc in range(NDT):
                    nc.tensor.matmul(
                        ps2, wkv[:, kc, m * P:(m + 1) * P],
                        eob_all[kc][:, TOWN:LE], start=kc == 0, stop=kc == NDT - 1,
                    )
                bcol = p_bias.tile([P, 1], f32, tag="bcol")
                nc.sync.dma_start(out=bcol, in_=db_caqkv[l][D + m * P:D + (m + 1) * P, :])
                nc.scalar.activation(caK[m][:, 0:TOWN], ps, AF.Identity, bias=bcol)
                nc.scalar.activation(caK[m][:, TOWN:LE], ps2, AF.Identity, bias=bcol)
            return caK

        def ca_v_project(l):
            wkv = load_w(p_w, dw_caqkv[l][:, 2 * D:3 * D], NDT, D, "w")
            brow512 = p_bias.tile([1, TOWN], bf16, tag="brow512")
            nc.sync.dma_start(
                out=brow512,
                in_=db_caqkv_bf[l][2 * D:3 * D, :].rearrange("a b -> b a"),
            )
            caV = alloc_vs()
            for tt in range(NKT):
                ps = ps_alt(tt)
                for kc in range(NDT):
                    nc.tensor.matmul(
                        ps, eob_all[kc][:, tt * P:(tt + 1) * P],
                        wkv[:, kc, 0:D],
                        start=kc == 0, stop=False,
                    )
                nc.tensor.matmul(ps, ones_row_bf, brow512, start=False, stop=True)
                nc.vector.tensor_copy(
                    caV[tt][:, :, 0:64],
                    ps.rearrange("p (h d) -> p h d", h=H),
                )
            return caV

        for l in range(n_dec):
            wq = load_w(p_w, dw_saqkv[l][:, 0:D], NDT, D, "w")
            wkv_ca = load_w(p_w, dw_caqkv[l][:, D:2 * D], NDT, D, "w")
            wk = load_w(p_w, dw_saqkv[l][:, D:2 * D], NDT, D, "w")
            wv = load_w(p_w, dw_saqkv[l][:, 2 * D:3 * D], NDT, D, "w")
            wout = load_w(p_w, dw_saout[l], NDT, D, "w")

            # self-attention (causal)
            hs = layer_norm(ys, bf16, p_h, "h")
            hbout = ag_h(hs, f"d{l}")
            Qs = proj_fm(wq, 0, 4, hs, db_saqkv[l], p_q, "q")
            # independent work to cover the AllGather flight:
            caK = ca_k_project(l, wkv_ca)
            Ks = [p_kv.tile([P, LE], bf16, tag="ksb", name=f"ks{m}") for m in range(NDT)]
            Vs = alloc_vs()
            hhalves = load_h_halves(hbout)
            for hh in range(2):
                kv_project(wk, wv, hhalves[hh], hh, db_saqkv[l], db_saqkv_bf[l], Ks, Vs)
            OTs = attention(Qs, Ks, Vs, all_kt, sa_masks, f"s{l}")
            ys = proj_residual(wout, 0, NDT, OTs, db_saout[l], ys)
            caV = ca_v_project(l)  # fills the LN2-chain PE bubble

            # cross-attention (no mask)
            wcaq = load_w(p_w, dw_caqkv[l][:, 0:D], NDT, D, "w")
            wcao = load_w(p_w, dw_caout[l], NDT, D, "w")
            hs = layer_norm(ys, bf16, p_h, "h")
            Qs = proj_fm(wcaq, 0, 4, hs, db_caqkv[l], p_q, "q")
            OTs = attention(Qs, caK, caV, list(range(NKT)), {}, f"c{l}")
            ys = proj_residual(wcao, 0, NDT, OTs, db_caout[l], ys)

            # ffn
            hs = layer_norm(ys, bf16, p_h, "h")
            ys = ffn(dw_f1[l], dw_f2[l], db_f1[l], db_f2[l], hs, ys)

        dof = layer_norm(ys, f32, p_eo, "eof")
        for dt in range(NDT):
            nc.sync.dma_start(out=dec_part[dt], in_=dof[dt])

    nc.compile()
    return nc


def make_in_maps(inputs):
    inp = {k: np.asarray(v) for k, v in inputs.items()}
    f32 = np.float32

    W = {
        "ew_qkv": np.ascontiguousarray(inp["e_qkv_w"].swapaxes(1, 2)).astype(BF16),
        "ew_out": np.ascontiguousarray(inp["e_out_w"].swapaxes(1, 2)).astype(BF16),
        "ew_f1": np.ascontiguousarray(inp["e_ff1_w"].swapaxes(1, 2)).astype(BF16),
        "ew_f2": np.ascontiguousarray(inp["e_ff2_w"].swapaxes(1, 2)).astype(BF16),
        "eb_qkv": inp["e_qkv_b"].astype(f32).reshape(NE, 3 * D, 1),
        "eb_out": inp["e_out_b"].astype(f32).reshape(NE, D, 1),
        "eb_f1": inp["e_ff1_b"].astype(f32).reshape(NE, F, 1),
        "eb_f2": inp["e_ff2_b"].astype(f32).reshape(NE, D, 1),
        "dw_saqkv": np.ascontiguousarray(inp["d_sa_qkv_w"].swapaxes(1, 2)).astype(BF16),
        "db_saqkv": inp["d_sa_qkv_b"].astype(f32).reshape(ND, 3 * D, 1),
        "dw_saout": np.ascontiguousarray(inp["d_sa_out_w"].swapaxes(1, 2)).astype(BF16),
        "db_saout": inp["d_sa_out_b"].astype(f32).reshape(ND, D, 1),
        "dw_caqkv": np.ascontiguousarray(inp["d_ca_qkv_w"].swapaxes(1, 2)).astype(BF16),
        "db_caqkv": inp["d_ca_qkv_b"].astype(f32).reshape(ND, 3 * D, 1),
        "dw_caout": np.ascontiguousarray(inp["d_ca_out_w"].swapaxes(1, 2)).astype(BF16),
        "db_caout": inp["d_ca_out_b"].astype(f32).reshape(ND, D, 1),
        "dw_f1": np.ascontiguousarray(inp["d_ff1_w"].swapaxes(1, 2)).astype(BF16),
        "db_f1": inp["d_ff1_b"].astype(f32).reshape(ND, F, 1),
        "dw_f2": np.ascontiguousarray(inp["d_ff2_w"].swapaxes(1, 2)).astype(BF16),
        "db_f2": inp["d_ff2_b"].astype(f32).reshape(ND, D, 1),
        "eb_qkv_bf": inp["e_qkv_b"].astype(BF16).reshape(NE, 3 * D, 1),
        "db_saqkv_bf": inp["d_sa_qkv_b"].astype(BF16).reshape(ND, 3 * D, 1),
        "db_caqkv_bf": inp["d_ca_qkv_b"].astype(BF16).reshape(ND, 3 * D, 1),
    }

    in_maps = []
    for c in range(NCORE):
        b, half = c // 2, c % 2
        sl = slice(half * TOWN, (half + 1) * TOWN)
        m = dict(W)
        xT = np.ascontiguousarray(inp["enc_in"][b].astype(f32).T[:, sl])
        m["x0T"] = xT.reshape(NDT, P, TOWN)
        yT = np.ascontiguousarray(inp["dec_in"][b].astype(f32).T[:, sl])
        m["y0T"] = yT.reshape(NDT, P, TOWN)
        xyz = inp["enc_xyz"][b].astype(f32)
        n2 = (xyz * xyz).sum(-1, dtype=f32).astype(f32)
        xq2 = (np.float32(2.0) * xyz[sl]).astype(f32)
        m["xq2"] = np.ascontiguousarray(xq2)
        xkn = np.concatenate([xyz, n2[:, None]], 1).astype(f32)
        m["xrow"] = np.ascontiguousarray(xkn.T)
        bos = np.full((TOWN, 1), 1e30, f32)
        if half == 0:
            bos[0, 0] = NEG
        m["boscol"] = bos
        m["eye"] = np.eye(P, dtype=np.float32)
        # causal tiles vs own queries, absolute key-tile order
        qg = np.arange(half * TOWN, (half + 1) * TOWN)
        kg = np.arange(LE)
        m["causal"] = np.ascontiguousarray(
            (kg[:, None] <= qg[None, :]).astype(BF16)
        ).reshape(NKT, P, TOWN)
        in_maps.append(m)
    return in_maps


def assemble(results):
    enc = np.zeros((B, LE, D), np.float32)
    dec = np.zeros((B, LD, D), np.float32)
    for c, r in enumerate(results):
        b, half = c // 2, c % 2
        sl = slice(half * TOWN, (half + 1) * TOWN)
        enc[b, sl, :] = r["enc_part"].reshape(D, TOWN).T
        dec[b, sl, :] = r["dec_part"].reshape(D, TOWN).T
    return enc, dec


def kernel(**inputs):
    from concourse import bass_utils

    if "nc" not in _CACHE:
        _CACHE["nc"] = build()
    nc = _CACHE["nc"]
    in_maps = make_in_maps(inputs)
    res = bass_utils.run_bass_kernel_spmd(
        nc, in_maps, core_ids=list(range(NCORE))
    )
    return assemble(res.results)


# revision 41
# speedup vs baseline: 1.0511x; 1.0511x over previous
"""Trainium2 Bass kernel for nn_EncoderDecoderTransformer (sparse kNN encoder attention).

Sharding: data-parallel over batch (4 batches x 2 cores); each pair of cores
splits the sequence (512 tokens each). Per attention sub-layer the pair
AllGathers the LN output h (bf16, 512KB) right after layer norm; each core then
projects K/V for the FULL sequence locally (PE has headroom), so the collective
is off the critical path (Q/K/V-own projections overlap the flight).
Cross-attention K/V are projected per decoder layer from a one-time AllGather
of enc_out - no per-layer cross collectives.

Layouts (per core):
  - Activations feature-major: x^T stored as 4 tiles (128 dims, 512 own tokens).
  - Q^T feature-major (head h lives in rows [64*(h%2):...] of ptile h//2).
  - K^T feature-major full-seq: 4 tiles [128, 1024] (columns = absolute token).
  - V token-major full-seq: 8 tiles (128 tokens, 8 heads, 65) with a constant-1
    column per head so the AV matmul also produces the softmax denominator.
  - Scores transposed: S^T = K^T.T @ Q^T, with BOTH heads of a ptile batched
    into one [128, 1024] PSUM tile so the Exp activation runs at N=1024
    (amortizes the ~352-cycle ACT overhead). kNN/causal masking is a 0/1 bf16
    multiply with column-duplicated [128,1024] mask tiles.
  - Decoder self-attention skips fully-masked key tiles (half-0 cores do 4 of
    8 kt) and skips the mask multiply on fully-allowed tiles (half-1, kt 0-3).
  - kNN mask: s'_qk = 2 x_q.x_k - |x_k|^2 orders like -distance; rank-17
    threshold via DVE max8/match_replace. Same math as the verified baseline.
"""

import os
import numpy as np
import ml_dtypes

BF16 = ml_dtypes.bfloat16

D, F, H, NE, ND, KNN = 512, 2048, 8, 4, 4, 16
B, LE, LD = 4, 1024, 1024
DH = D // H
NCORE = 8
P = 128
TOWN = 512          # tokens owned per core
T2 = 2 * TOWN       # batched free dim (two heads side by side)
NDT = D // P        # 4 feature tiles
NKT = LE // P       # 8 key tiles
NEG = -1e30
EPS = 1e-5
PAIRS = [[0, 1], [2, 3], [4, 5], [6, 7]]

_CACHE = {}


def build(n_enc=NE, n_dec=ND):
    from contextlib import ExitStack

    import concourse.bacc as bacc
    import concourse.tile as tile
    import concourse.mybir as mybir

    f32 = mybir.dt.float32
    bf16 = mybir.dt.bfloat16
    AF = mybir.ActivationFunctionType
    OP = mybir.AluOpType

    nc = bacc.Bacc("TRN2", target_bir_lowering=False, debug=False, num_devices=NCORE)

    # ---- I/O ----
    def din(name, shape, dt=f32):
        return nc.dram_tensor(name, shape, dt, kind="ExternalInput")

    x0T = din("x0T", [NDT, P, TOWN])
    y0T = din("y0T", [NDT, P, TOWN])
    xq2_d = din("xq2", [TOWN, 3])       # 2*xyz for own tokens
    xrow_d = din("xrow", [4, LE])        # [xyz, |xyz|^2] all tokens, transposed
    boscol_d = din("boscol", [TOWN, 1])  # threshold override col (NEG at BOS q)
    eye_in = din("eye", [P, P])
    causal_in = din("causal", [NKT, P, TOWN], bf16)  # per-core causal kt tiles

    ew_qkv = din("ew_qkv", [NE, D, 3 * D], bf16)
    ew_out = din("ew_out", [NE, D, D], bf16)
    ew_f1 = din("ew_f1", [NE, D, F], bf16)
    ew_f2 = din("ew_f2", [NE, F, D], bf16)
    eb_qkv = din("eb_qkv", [NE, 3 * D, 1])
    eb_out = din("eb_out", [NE, D, 1])
    eb_f1 = din("eb_f1", [NE, F, 1])
    eb_f2 = din("eb_f2", [NE, D, 1])

    dw_saqkv = din("dw_saqkv", [ND, D, 3 * D], bf16)
    db_saqkv = din("db_saqkv", [ND, 3 * D, 1])
    dw_saout = din("dw_saout", [ND, D, D], bf16)
    db_saout = din("db_saout", [ND, D, 1])
    dw_caqkv = din("dw_caqkv", [ND, D, 3 * D], bf16)
    db_caqkv = din("db_caqkv", [ND, 3 * D, 1])
    dw_caout = din("dw_caout", [ND, D, D], bf16)
    db_caout = din("db_caout", [ND, D, 1])
    dw_f1 = din("dw_f1", [ND, D, F], bf16)
    db_f1 = din("db_f1", [ND, F, 1])
    dw_f2 = din("dw_f2", [ND, F, D], bf16)
    db_f2 = din("db_f2", [ND, D, 1])
    eb_qkv_bf = din("eb_qkv_bf", [NE, 3 * D, 1], bf16)
    db_saqkv_bf = din("db_saqkv_bf", [ND, 3 * D, 1], bf16)
    db_caqkv_bf = din("db_caqkv_bf", [ND, 3 * D, 1], bf16)

    enc_part = nc.dram_tensor("enc_part", [NDT, P, TOWN], f32, kind="ExternalOutput")
    dec_part = nc.dram_tensor("dec_part", [NDT, P, TOWN], f32, kind="ExternalOutput")

    with tile.TileContext(nc) as tc, ExitStack() as ctx:
        ep = ctx.enter_context

        pc = ep(tc.tile_pool(name="pc", bufs=1))
        p_allow = ep(tc.tile_pool(name="p_allow", bufs=8))
        p_causal = ep(tc.tile_pool(name="p_causal", bufs=8))
        ps_s = ep(tc.tile_pool(name="ps_s", bufs=2, space="PSUM"))
        ps_o = ep(tc.tile_pool(name="ps_o", bufs=2, space="PSUM"))
        ps_mm = ep(tc.tile_pool(name="ps_mm", bufs=2, space="PSUM"))
        p_dram = ep(tc.tile_pool(name="p_dram", bufs=2, space="DRAM"))

        # ---- constants ----
        ones_col_bf = pc.tile([P, 1], bf16)
        nc.vector.memset(ones_col_bf, 1.0)
        ones_row = pc.tile([1, P], f32)
        nc.vector.memset(ones_row, 1.0)
        ones_row_bf = pc.tile([1, P], bf16)
        nc.vector.memset(ones_row_bf, 1.0)
        eps_sb = pc.tile([1, 1], f32)
        nc.vector.memset(eps_sb, EPS)
        epsD_sb = pc.tile([1, 1], f32)
        nc.vector.memset(epsD_sb, float(D) * D * EPS)
        lnD_sb = pc.tile([1, 1], f32)
        nc.vector.memset(lnD_sb, float(np.log(D)))
        dummy_w = pc.tile([P, P], bf16)
        nc.vector.memset(dummy_w, 0.0)
        dummy_x = pc.tile([P, TOWN], bf16)
        nc.vector.memset(dummy_x, 0.0)

        eye_f32 = pc.tile([P, P], f32)
        nc.sync.dma_start(out=eye_f32, in_=eye_in[:, :])

        causal_sb = []
        for i in range(NKT):
            t = p_causal.tile([P, TOWN], bf16, tag="causal")
            nc.sync.dma_start(out=t, in_=causal_in[i])
            causal_sb.append(t)

        def pe_warm(n):
            psD = ps_mm.tile([P, TOWN], f32, tag="mm")
            for _ in range(n):
                nc.tensor.matmul(psD, dummy_w, dummy_x, start=True, stop=True)

        def build_mask():
            # Exact-fp32 kNN scores s3 = 2 x_q.x_k - |x_k|^2 (orders like
            # -distance), computed query-major once; the rank-17 value (16 NN +
            # self) is the inclusion threshold. allow = (s3 >= thr) compares the
            # SAME fp32 values the threshold came from, then the 0/1 bf16 mask
            # is moved to key-major via exact PE transposes.
            import concourse.bass as cbass

            def bcast_rows(dram_row_ap, pool, n_free, tag):
                t = pool.tile([P, n_free], f32, tag=tag)
                src_ap = cbass.AP(
                    tensor=dram_row_ap.tensor, offset=dram_row_ap.offset,
                    ap=[[0, P]] + list(dram_row_ap.ap),
                )
                nc.sync.dma_start(out=t, in_=src_ap)
                return t

            allow_sb = []
            for kt in range(NKT):
                t = p_allow.tile([P, T2], bf16, tag="allow", name=f"allow{kt}")
                allow_sb.append(t)
            with tc.tile_pool(name="p_mask", bufs=3) as p_mask, \
                 tc.tile_pool(name="p_mbc", bufs=1) as p_mbc, \
                 tc.tile_pool(name="p_m8", bufs=8) as p_m8, \
                 tc.tile_pool(name="p_alq", bufs=2) as p_alq:
                bcx = []
                for c in range(4):
                    t = bcast_rows(xrow_d[c], p_mbc, LE, tag=f"bcx{c}")
                    bcx.append(t)
                for qt in range(4):
                    xqc = p_m8.tile([P, 3], f32, tag="xqc")
                    nc.sync.dma_start(out=xqc, in_=xq2_d[qt * P:(qt + 1) * P, :])
                    bosc = p_m8.tile([P, 1], f32, tag="bosc")
                    nc.sync.dma_start(out=bosc, in_=boscol_d[qt * P:(qt + 1) * P, :])
                    s0 = p_mask.tile([P, LE], f32, tag="s")
                    nc.vector.tensor_scalar(s0, bcx[0], xqc[:, 0:1], None, op0=OP.mult)
                    s1 = p_mask.tile([P, LE], f32, tag="s")
                    nc.vector.scalar_tensor_tensor(s1, bcx[1], xqc[:, 1:2], s0, OP.mult, OP.add)
                    s2 = p_mask.tile([P, LE], f32, tag="s")
                    nc.vector.scalar_tensor_tensor(s2, bcx[2], xqc[:, 2:3], s1, OP.mult, OP.add)
                    s3 = p_mask.tile([P, LE], f32, tag="s")
                    nc.vector.tensor_tensor(s3, s2, bcx[3], OP.subtract)
                    psW = ps_s.tile([P, T2], f32, tag="pss")
                    nc.tensor.matmul(psW[:, 0:TOWN], s3[:, 0:P], bcx[0][:, 0:TOWN],
                                     start=True, stop=True)
                    m8 = p_m8.tile([P, 8], f32, tag="m8")
                    nc.vector.max(m8, s3)
                    s4 = p_mask.tile([P, LE], f32, tag="s")
                    nc.vector.match_replace(s4, m8, s3, NEG)
                    m8b = p_m8.tile([P, 8], f32, tag="m8")
                    nc.vector.max(m8b, s4)
                    s5 = p_mask.tile([P, LE], f32, tag="s")
                    nc.vector.match_replace(s5, m8b, s4, NEG)
                    nc.tensor.matmul(psW[:, TOWN:T2], s5[:, 0:P], bcx[0][:, 0:TOWN],
                                     start=True, stop=True)
                    m8c = p_m8.tile([P, 8], f32, tag="m8")
                    nc.vector.max(m8c, s5)
                    thr = p_m8.tile([P, 1], f32, tag="thr")
                    nc.vector.tensor_tensor(thr, m8c[:, 0:1], bosc, OP.min)
                    al_q = p_alq.tile([P, LE], f32, tag="alq")
                    nc.vector.tensor_scalar(al_q, s3, thr, None, op0=OP.is_ge)
                    for kt in range(NKT):
                        psT = ps_mm.tile([P, P], f32, tag="mm")
                        nc.tensor.transpose(psT, al_q[:, kt * P:(kt + 1) * P], eye_f32)
                        nc.vector.tensor_copy(
                            allow_sb[kt][:, qt * P:(qt + 1) * P], psT
                        )
                for kt in range(NKT):
                    nc.vector.tensor_copy(allow_sb[kt][:, TOWN:T2], allow_sb[kt][:, 0:TOWN])
                nc.vector.memset(allow_sb[0][0:1, :], 1.0)  # BOS key allowed for all q
            return allow_sb

        pe_warm(12)
        allow_sb = build_mask()

        # ================= helpers =================
        def load_w(pool, dram_ap, kchunks, cols, tag):
            t = pool.tile([P, kchunks, cols], bf16, tag=tag)
            nc.sync.dma_start(
                out=t, in_=dram_ap.rearrange("(kc p) m -> p kc m", p=P)
            )
            return t

        def ps_alt(i):
            if i % 2 == 0:
                return ps_mm.tile([P, TOWN], f32, tag="mm", name=f"psa{i}")
            return ps_s.tile([P, TOWN], f32, tag="pss", name=f"psb{i}")

        def layer_norm(xs, out_dt, out_pool, out_tag):
            sqs = []
            xbs = []
            for dt in range(NDT):
                sq = p_lnsq.tile([P, TOWN], bf16, tag="lnsq")
                nc.scalar.activation(sq, xs[dt], AF.Square)
                sqs.append(sq)
                xb = p_lnsq.tile([P, TOWN], bf16, tag="lnxb")
                nc.vector.tensor_copy(xb, xs[dt])
                xbs.append(xb)
            ps_mean = ps_mm.tile([1, TOWN], f32, tag="mm")
            for dt in range(NDT):
                nc.tensor.matmul(ps_mean, ones_col_bf, xbs[dt], start=dt == 0, stop=dt == 3)
            ps_sq = ps_mm.tile([1, TOWN], f32, tag="mm")
            for dt in range(NDT):
                nc.tensor.matmul(ps_sq, ones_col_bf, sqs[dt], start=dt == 0, stop=dt == 3)
            # v' = D*sum(x^2) - sum(x)^2 = D^2*var; rstd = exp(-.5 ln(v'+D^2 eps) + ln D)
            psW = ps_s.tile([P, T2], f32, tag="pss")
            mu = p_small.tile([1, TOWN], f32, tag="sm")
            nc.vector.tensor_single_scalar(mu, ps_mean, 1.0 / D, OP.mult)
            s1sq = p_small.tile([1, TOWN], f32, tag="sm")
            nc.scalar.activation(s1sq, ps_mean, AF.Square)
            nc.tensor.matmul(psW[:, 0:TOWN], ones_row, s1sq, start=True, stop=True)
            var = p_small.tile([1, TOWN], f32, tag="sm")
            nc.vector.scalar_tensor_tensor(var, ps_sq, float(D), s1sq, OP.mult, OP.subtract)
            nc.tensor.matmul(psW[:, 0:TOWN], ones_row, var, start=True, stop=True)
            lnv = p_small.tile([1, TOWN], f32, tag="sm")
            nc.scalar.activation(lnv, var, AF.Ln, bias=epsD_sb)
            nc.tensor.matmul(psW[:, 0:TOWN], ones_row, lnv, start=True, stop=True)
            rstd = p_small.tile([1, TOWN], f32, tag="sm")
            nc.scalar.activation(rstd, lnv, AF.Exp, scale=-0.5, bias=lnD_sb)
            nc.tensor.matmul(psW[:, 0:TOWN], ones_row, rstd, start=True, stop=True)
            cro = p_small.tile([1, TOWN], f32, tag="sm")
            nc.vector.scalar_tensor_tensor(cro, mu, -1.0, rstd, OP.mult, OP.mult)
            ps_a = ps_mm.tile([P, TOWN], f32, tag="mm")
            nc.tensor.matmul(ps_a, ones_row, rstd, start=True, stop=True)
            ps_c = ps_mm.tile([P, TOWN], f32, tag="mm")
            nc.tensor.matmul(ps_c, ones_row, cro, start=True, stop=True)
            a_sb = p_lnac.tile([P, TOWN], f32, tag="lna")
            nc.vector.tensor_copy(a_sb, ps_a)
            c_sb = p_lnac.tile([P, TOWN], f32, tag="lnc")
            nc.vector.tensor_copy(c_sb, ps_c)
            hs = []
            for dt in range(NDT):
                h = out_pool.tile([P, TOWN], out_dt, tag=out_tag)
                nc.vector.tensor_tensor(h, xs[dt], a_sb, OP.mult)
                nc.vector.tensor_tensor(h, h, c_sb, OP.add)
                hs.append(h)
            return hs

        def proj_fm(w_sb, col_off, n_m, rhs, bias_ap, out_pool, out_tag, out_dt=bf16):
            """Feature-major projection; per-partition bias applied on eviction."""
            outs = []
            nk = len(rhs)
            for m in range(n_m):
                ps = ps_alt(m)
                for kc in range(nk):
                    nc.tensor.matmul(
                        ps, w_sb[:, kc, col_off + m * P:col_off + (m + 1) * P],
                        rhs[kc], start=kc == 0, stop=kc == nk - 1,
                    )
                bcol = p_bias.tile([P, 1], f32, tag="bcol")
                nc.sync.dma_start(out=bcol, in_=bias_ap[col_off + m * P:col_off + (m + 1) * P, :])
                o = out_pool.tile([P, TOWN], out_dt, tag=out_tag)
                nc.vector.tensor_scalar(o, ps, bcol, None, op0=OP.add)
                outs.append(o)
            return outs

        def kv_project(wk_sb, wv_sb, hs_half, hh, bias_ap, bias_bf_ap, Ks, Vs):
            """Project K (feature-major, into Ks column half hh) and V (token-
            major tiles Vs[4*hh + tt]) for the 512 tokens of half hh.
            bias_ap/bias_bf_ap hold [3D] packed qkv bias; K at +D, V at +2D."""
            csl = slice(hh * TOWN, (hh + 1) * TOWN)
            for m in range(NDT):
                ps = ps_alt(m)
                for kc in range(NDT):
                    nc.tensor.matmul(
                        ps, wk_sb[:, kc, m * P:(m + 1) * P],
                        hs_half[kc], start=kc == 0, stop=kc == NDT - 1,
                    )
                bcol = p_bias.tile([P, 1], f32, tag="bcol")
                nc.sync.dma_start(out=bcol, in_=bias_ap[D + m * P:D + (m + 1) * P, :])
                nc.scalar.activation(Ks[m][:, csl], ps, AF.Identity, bias=bcol)
            brow512 = p_bias.tile([1, TOWN], bf16, tag="brow512")
            nc.sync.dma_start(
                out=brow512,
                in_=bias_bf_ap[2 * D:3 * D, :].rearrange("a b -> b a"),
            )
            for tt in range(4):
                ps = ps_alt(tt)
                for kc in range(NDT):
                    nc.tensor.matmul(
                        ps, hs_half[kc][:, tt * P:(tt + 1) * P],
                        wv_sb[:, kc, 0:D],
                        start=kc == 0, stop=False,
                    )
                nc.tensor.matmul(ps, ones_row_bf, brow512, start=False, stop=True)
                vt = Vs[4 * hh + tt]
                nc.vector.tensor_copy(
                    vt[:, :, 0:64],
                    ps.rearrange("p (h d) -> p h d", h=H),
                )

        def alloc_vs():
            Vs = []
            for tt in range(NKT):
                vt = p_v.tile([P, H, 65], bf16, tag="vsb")
                nc.vector.memset(vt[:, :, 64:65], 1.0)
                Vs.append(vt)
            return Vs

        def attention(Qs, Ks, Vs, kt_order, masks, uniq):
            """masks: dict kt -> ("dup", [P,T2] tile) | ("half", [P,TOWN] tile).
            S matmuls run two kt ahead of AV (PE never stalls on exp/mask);
            the psO eviction chain of head-pair hp is emitted after the next
            head-pair's first S matmuls so its rec-wait overlaps real PE work."""
            OTs = []
            nkt = len(kt_order)
            pending = []

            def emit_evict(psO, ot):
                for j in range(2):
                    den = p_small.tile([1, TOWN], f32, tag="sm", name=f"den{j}")
                    nc.vector.tensor_copy(den, psO[j][64:65, :])
                    rec = p_small.tile([1, TOWN], f32, tag="sm", name=f"rec{j}")
                    nc.vector.reciprocal_approx_fast(rec, den)
                    psB = ps_mm.tile([64, TOWN], f32, tag="mm", name=f"psB{j}")
                    nc.tensor.matmul(psB, ones_row[:, 0:64], rec, start=True, stop=True)
                    bc = p_bc.tile([64, TOWN], f32, tag="bc", name=f"bc{j}")
                    nc.vector.tensor_copy(bc, psB)
                    nc.vector.tensor_tensor(ot[j * 64:(j + 1) * 64, :], psO[j][0:64, :], bc, OP.mult)

            for hp in range(4):
                psO = []
                for _j in range(2):
                    psO_t = ps_o.tile([65, TOWN], f32, tag="pso")
                    psO.append(psO_t)
                e2s = {}

                def emit_S(idx, hp=hp, e2s=e2s):
                    kt = kt_order[idx]
                    psS = ps_s.tile([P, T2], f32, tag="pss")
                    for j in range(2):
                        nc.tensor.matmul(
                            psS[:, j * TOWN:(j + 1) * TOWN],
                            Ks[hp][j * 64:(j + 1) * 64, kt * P:(kt + 1) * P],
                            Qs[hp][j * 64:(j + 1) * 64, :],
                            start=True, stop=True,
                        )
                    e = p_e.tile([P, T2], bf16, tag="e")
                    nc.scalar.activation(e, psS, AF.Exp, scale=0.125)
                    m = masks.get(kt)
                    if m is not None:
                        kind, mt = m
                        e2 = p_e.tile([P, T2], bf16, tag="e")
                        if kind == "dup":
                            nc.vector.tensor_tensor(e2, e, mt, OP.mult)
                        else:
                            for j in range(2):
                                jsl = slice(j * TOWN, (j + 1) * TOWN)
                                nc.vector.tensor_tensor(e2[:, jsl], e[:, jsl], mt, OP.mult)
                    else:
                        e2 = e
                    e2s[idx] = e2

                def emit_AV(idx, hp=hp, e2s=e2s, psO=psO):
                    kt = kt_order[idx]
                    e2 = e2s.pop(idx)
                    for j in range(2):
                        nc.tensor.matmul(
                            psO[j], Vs[kt][:, 2 * hp + j, :],
                            e2[:, j * TOWN:(j + 1) * TOWN],
                            start=idx == 0, stop=idx == nkt - 1,
                        )

                emit_S(0)
                if nkt > 1:
                    emit_S(1)
                if pending:
                    emit_evict(*pending.pop())
                for idx in range(nkt):
                    emit_AV(idx)
                    if idx + 2 < nkt:
                        emit_S(idx + 2)
                ot = p_ot.tile([P, TOWN], bf16, tag="ot")
                pending.append((psO, ot))
                OTs.append(ot)
            emit_evict(*pending.pop())
            return OTs

        def proj_residual(w_sb, col_off, n_k, rhs, bias_ap, xs):
            nxs = []
            for m in range(NDT):
                ps = ps_alt(m)
                for kc in range(n_k):
                    nc.tensor.matmul(
                        ps, w_sb[:, kc, col_off + m * P:col_off + (m + 1) * P],
                        rhs[kc], start=kc == 0, stop=kc == n_k - 1,
                    )
                bcol = p_bias.tile([P, 1], f32, tag="bcol")
                nc.sync.dma_start(out=bcol, in_=bias_ap[m * P:(m + 1) * P, :])
                nx = p_x.tile([P, TOWN], f32, tag="x")
                nc.vector.scalar_tensor_tensor(nx, ps, bcol, xs[m], OP.add, OP.add)
                nxs.append(nx)
            return nxs

        def ffn(w1_ap, w2_ap, b1_ap, b2_ap, hs, xs):
            gs = []
            for m in range(F // P):
                w1m = p_w1.tile([P, NDT, P], bf16, tag="wf1")
                nc.sync.dma_start(
                    out=w1m,
                    in_=w1_ap[:, m * P:(m + 1) * P].rearrange("(kc p) m -> p kc m", p=P),
                )
                ps = ps_alt(m)
                for kc in range(NDT):
                    nc.tensor.matmul(
                        ps, w1m[:, kc, :], hs[kc],
                        start=kc == 0, stop=kc == NDT - 1,
                    )
                bcol = p_bias.tile([P, 1], f32, tag="bcol")
                nc.sync.dma_start(out=bcol, in_=b1_ap[m * P:(m + 1) * P, :])
                g = p_g.tile([P, TOWN], bf16, tag="g")
                nc.scalar.activation(g, ps, AF.Gelu, bias=bcol)
                gs.append(g)
            nxs = []
            for m in range(NDT):
                w2m = p_w2.tile([P, F // P, P], bf16, tag="wf2")
                nc.sync.dma_start(
                    out=w2m,
                    in_=w2_ap[:, m * P:(m + 1) * P].rearrange("(kc p) c -> p kc c", p=P),
                )
                ps2 = ps_alt(m)
                for kc in range(F // P):
                    nc.tensor.matmul(
                        ps2, w2m[:, kc, :], gs[kc],
                        start=kc == 0, stop=kc == F // P - 1,
                    )
                bcol = p_bias.tile([P, 1], f32, tag="bcol")
                nc.sync.dma_start(out=bcol, in_=b2_ap[m * P:(m + 1) * P, :])
                nx = p_x.tile([P, TOWN], f32, tag="x")
                nc.vector.scalar_tensor_tensor(nx, ps2, bcol, xs[m], OP.add, OP.add)
                nxs.append(nx)
            return nxs

        def ag_h(hs, uniq):
            """DMA h tiles to a DRAM bounce and AllGather across the pair.
            Returns the gathered [2, NDT, P, TOWN] DRAM tile (index = half)."""
            hbin = p_dram.tile([NDT, P, TOWN], bf16, tag=f"hbin{uniq}")
            for dt in range(NDT):
                nc.sync.dma_start(out=hbin[dt], in_=hs[dt])
            hbout = p_dram.tile([2, NDT, P, TOWN], bf16, tag=f"hbout{uniq}")
            nc.gpsimd.collective_compute(
                "AllGather", OP.bypass, replica_groups=PAIRS,
                ins=[hbin[:].opt()], outs=[hbout[:].opt()],
            )
            pe_warm(10)
            return hbout

        def load_h_halves(hbout):
            halves = []
            for hh in range(2):
                tiles = []
                for dt in range(NDT):
                    t = p_h.tile([P, TOWN], bf16, tag="hall")
                    nc.sync.dma_start(out=t, in_=hbout[hh, dt])
                    tiles.append(t)
                halves.append(tiles)
            return halves

        p_x = ep(tc.tile_pool(name="p_x", bufs=5))
        p_h = ep(tc.tile_pool(name="p_h", bufs=12))
        p_q = ep(tc.tile_pool(name="p_q", bufs=5))
        p_kv = ep(tc.tile_pool(name="p_kv", bufs=8))
        p_v = ep(tc.tile_pool(name="p_v", bufs=9))
        p_ot = ep(tc.tile_pool(name="p_ot", bufs=4))
        p_e = ep(tc.tile_pool(name="p_e", bufs=4))
        p_g = ep(tc.tile_pool(name="p_g", bufs=16))
        p_lnsq = ep(tc.tile_pool(name="p_lnsq", bufs=4))
        p_lnac = ep(tc.tile_pool(name="p_lnac", bufs=2))
        p_bc = ep(tc.tile_pool(name="p_bc", bufs=2))
        p_small = ep(tc.tile_pool(name="p_small", bufs=6))
        p_bias = ep(tc.tile_pool(name="p_bias", bufs=4))
        p_eo = ep(tc.tile_pool(name="p_eo", bufs=2))
        p_eoball = ep(tc.tile_pool(name="p_eoball", bufs=4))
        p_w1 = ep(tc.tile_pool(name="p_w1", bufs=2))
        p_w2 = ep(tc.tile_pool(name="p_w2", bufs=2))
        p_w = ep(tc.tile_pool(name="p_w", bufs=3))

        all_kt = list(range(NKT))
        sa_masks = {kt: ("half", causal_sb[kt]) for kt in range(NKT)}
        enc_masks = {kt: ("dup", allow_sb[kt]) for kt in range(NKT)}

        # ================= encoder =================
        xs = []
        for dt in range(NDT):
            x = p_x.tile([P, TOWN], f32, tag="x")
            nc.sync.dma_start(out=x, in_=x0T[dt])
            xs.append(x)

        for l in range(n_enc):
            wq = load_w(p_w, ew_qkv[l][:, 0:D], NDT, D, "w")
            wk = load_w(p_w, ew_qkv[l][:, D:2 * D], NDT, D, "w")
            wv = load_w(p_w, ew_qkv[l][:, 2 * D:3 * D], NDT, D, "w")
            wout = load_w(p_w, ew_out[l], NDT, D, "w")

            hs = layer_norm(xs, bf16, p_h, "h")
            hbout = ag_h(hs, f"e{l}")
            Qs = proj_fm(wq, 0, 4, hs, eb_qkv[l], p_q, "q")
            Ks = [p_kv.tile([P, LE], bf16, tag="ksb", name=f"ks{m}") for m in range(NDT)]
            Vs = alloc_vs()
            hhalves = load_h_halves(hbout)
            for hh in range(2):
                kv_project(wk, wv, hhalves[hh], hh, eb_qkv[l], eb_qkv_bf[l], Ks, Vs)
            OTs = attention(Qs, Ks, Vs, all_kt, enc_masks, f"e{l}")
            xs = proj_residual(wout, 0, NDT, OTs, eb_out[l], xs)
            hs = layer_norm(xs, bf16, p_h, "h")
            xs = ffn(ew_f1[l], ew_f2[l], eb_f1[l], eb_f2[l], hs, xs)

        eof = layer_norm(xs, f32, p_eo, "eof")
        eob = []
        for dt in range(NDT):
            nc.sync.dma_start(out=enc_part[dt], in_=eof[dt])
            t = p_h.tile([P, TOWN], bf16, tag="eob")
            nc.vector.tensor_copy(t, eof[dt])
            eob.append(t)
        ebout = ag_h(eob, "eo")
        eob_all = []
        for dt in range(NDT):
            t = p_eoball.tile([P, LE], bf16, tag="eoball")
            for hh in range(2):
                nc.sync.dma_start(
                    out=t[:, hh * TOWN:(hh + 1) * TOWN], in_=ebout[hh, dt]
                )
            eob_all.append(t)

        # ================= decoder =================
        ys = []
        for dt in range(NDT):
            y = p_x.tile([P, TOWN], f32, tag="x")
            nc.sync.dma_start(out=y, in_=y0T[dt])
            ys.append(y)

        def ca_k_project(l, wkv):
            caK = [p_kv.tile([P, LE], bf16, tag="ksb", name=f"cak{m}") for m in range(NDT)]
            for m in range(NDT):
                ps = ps_mm.tile([P, TOWN], f32, tag="mm")
                for kc in range(NDT):
                    nc.tensor.matmul(
                        ps, wkv[:, kc, m * P:(m + 1) * P],
                        eob_all[kc][:, 0:TOWN], start=kc == 0, stop=kc == NDT - 1,
                    )
                ps2 = ps_s.tile([P, TOWN], f32, tag="pss")
                for kc in range(NDT):
                    nc.tensor.matmul(
                        ps2, wkv[:, kc, m * P:(m + 1) * P],
                        eob_all[kc][:, TOWN:LE], start=kc == 0, stop=kc == NDT - 1,
                    )
                bcol = p_bias.tile([P, 1], f32, tag="bcol")
                nc.sync.dma_start(out=bcol, in_=db_caqkv[l][D + m * P:D + (m + 1) * P, :])
                nc.scalar.activation(caK[m][:, 0:TOWN], ps, AF.Identity, bias=bcol)
                nc.scalar.activation(caK[m][:, TOWN:LE], ps2, AF.Identity, bias=bcol)
            return caK

        def ca_v_project(l):
            wkv = load_w(p_w, dw_caqkv[l][:, 2 * D:3 * D], NDT, D, "w")
            brow512 = p_bias.tile([1, TOWN], bf16, tag="brow512")
            nc.sync.dma_start(
                out=brow512,
                in_=db_caqkv_bf[l][2 * D:3 * D, :].rearrange("a b -> b a"),
            )
            caV = alloc_vs()
            for tt in range(NKT):
                ps = ps_alt(tt)
                for kc in range(NDT):
                    nc.tensor.matmul(
                        ps, eob_all[kc][:, tt * P:(tt + 1) * P],
                        wkv[:, kc, 0:D],
                        start=kc == 0, stop=False,
                    )
                nc.tensor.matmul(ps, ones_row_bf, brow512, start=False, stop=True)
                nc.vector.tensor_copy(
                    caV[tt][:, :, 0:64],
                    ps.rearrange("p (h d) -> p h d", h=H),
                )
            return caV

        for l in range(n_dec):
            wq = load_w(p_w, dw_saqkv[l][:, 0:D], NDT, D, "w")
            wkv_ca = load_w(p_w, dw_caqkv[l][:, D:2 * D], NDT, D, "w")
            wk = load_w(p_w, dw_saqkv[l][:, D:2 * D], NDT, D, "w")
            wv = load_w(p_w, dw_saqkv[l][:, 2 * D:3 * D], NDT, D, "w")
            wout = load_w(p_w, dw_saout[l], NDT, D, "w")

            # self-attention (causal)
            hs = layer_norm(ys, bf16, p_h, "h")
            hbout = ag_h(hs, f"d{l}")
            Qs = proj_fm(wq, 0, 4, hs, db_saqkv[l], p_q, "q")
            # independent work to cover the AllGather flight:
            caK = ca_k_project(l, wkv_ca)
            Ks = [p_kv.tile([P, LE], bf16, tag="ksb", name=f"ks{m}") for m in range(NDT)]
            Vs = alloc_vs()
            hhalves = load_h_halves(hbout)
            for hh in range(2):
                kv_project(wk, wv, hhalves[hh], hh, db_saqkv[l], db_saqkv_bf[l], Ks, Vs)
            OTs = attention(Qs, Ks, Vs, all_kt, sa_masks, f"s{l}")
            ys = proj_residual(wout, 0, NDT, OTs, db_saout[l], ys)
            caV = ca_v_project(l)  # fills the LN2-chain PE bubble

            # cross-attention (no mask)
            wcaq = load_w(p_w, dw_caqkv[l][:, 0:D], NDT, D, "w")
            wcao = load_w(p_w, dw_caout[l], NDT, D, "w")
            hs = layer_norm(ys, bf16, p_h, "h")
            Qs = proj_fm(wcaq, 0, 4, hs, db_caqkv[l], p_q, "q")
            OTs = attention(Qs, caK, caV, list(range(NKT)), {}, f"c{l}")
            ys = proj_residual(wcao, 0, NDT, OTs, db_caout[l], ys)

            # ffn
            hs = layer_norm(ys, bf16, p_h, "h")
            ys = ffn(dw_f1[l], dw_f2[l], db_f1[l], db_f2[l], hs, ys)

        dof = layer_norm(ys, f32, p_eo, "eof")
        for dt in range(NDT):
            nc.sync.dma_start(out=dec_part[dt], in_=dof[dt])

    nc.compile()
    return nc


def make_in_maps(inputs):
    inp = {k: np.asarray(v) for k, v in inputs.items()}
    f32 = np.float32

    W = {
        "ew_qkv": np.ascontiguousarray(inp["e_qkv_w"].swapaxes(1, 2)).astype(BF16),
        "ew_out": np.ascontiguousarray(inp["e_out_w"].swapaxes(1, 2)).astype(BF16),
        "ew_f1": np.ascontiguousarray(inp["e_ff1_w"].swapaxes(1, 2)).astype(BF16),
        "ew_f2": np.ascontiguousarray(inp["e_ff2_w"].swapaxes(1, 2)).astype(BF16),
        "eb_qkv": inp["e_qkv_b"].astype(f32).reshape(NE, 3 * D, 1),
        "eb_out": inp["e_out_b"].astype(f32).reshape(NE, D, 1),
        "eb_f1": inp["e_ff1_b"].astype(f32).reshape(NE, F, 1),
        "eb_f2": inp["e_ff2_b"].astype(f32).reshape(NE, D, 1),
        "dw_saqkv": np.ascontiguousarray(inp["d_sa_qkv_w"].swapaxes(1, 2)).astype(BF16),
        "db_saqkv": inp["d_sa_qkv_b"].astype(f32).reshape(ND, 3 * D, 1),
        "dw_saout": np.ascontiguousarray(inp["d_sa_out_w"].swapaxes(1, 2)).astype(BF16),
        "db_saout": inp["d_sa_out_b"].astype(f32).reshape(ND, D, 1),
        "dw_caqkv": np.ascontiguousarray(inp["d_ca_qkv_w"].swapaxes(1, 2)).astype(BF16),
        "db_caqkv": inp["d_ca_qkv_b"].astype(f32).reshape(ND, 3 * D, 1),
        "dw_caout": np.ascontiguousarray(inp["d_ca_out_w"].swapaxes(1, 2)).astype(BF16),
        "db_caout": inp["d_ca_out_b"].astype(f32).reshape(ND, D, 1),
        "dw_f1": np.ascontiguousarray(inp["d_ff1_w"].swapaxes(1, 2)).astype(BF16),
        "db_f1": inp["d_ff1_b"].astype(f32).reshape(ND, F, 1),
        "dw_f2": np.ascontiguousarray(inp["d_ff2_w"].swapaxes(1, 2)).astype(BF16),
        "db_f2": inp["d_ff2_b"].astype(f32).reshape(ND, D, 1),
        "eb_qkv_bf": inp["e_qkv_b"].astype(BF16).reshape(NE, 3 * D, 1),
        "db_saqkv_bf": inp["d_sa_qkv_b"].astype(BF16).reshape(ND, 3 * D, 1),
        "db_caqkv_bf": inp["d_ca_qkv_b"].astype(BF16).reshape(ND, 3 * D, 1),
    }

    in_maps = []
    for c in range(NCORE):
        b, half = c // 2, c % 2
        sl = slice(half * TOWN, (half + 1) * TOWN)
        m = dict(W)
        xT = np.ascontiguousarray(inp["enc_in"][b].astype(f32).T[:, sl])
        m["x0T"] = xT.reshape(NDT, P, TOWN)
        yT = np.ascontiguousarray(inp["dec_in"][b].astype(f32).T[:, sl])
        m["y0T"] = yT.reshape(NDT, P, TOWN)
        xyz = inp["enc_xyz"][b].astype(f32)
        n2 = (xyz * xyz).sum(-1, dtype=f32).astype(f32)
        xq2 = (np.float32(2.0) * xyz[sl]).astype(f32)
        m["xq2"] = np.ascontiguousarray(xq2)
        xkn = np.concatenate([xyz, n2[:, None]], 1).astype(f32)
        m["xrow"] = np.ascontiguousarray(xkn.T)
        bos = np.full((TOWN, 1), 1e30, f32)
        if half == 0:
            bos[0, 0] = NEG
        m["boscol"] = bos
        m["eye"] = np.eye(P, dtype=np.float32)
        # causal tiles vs own queries, absolute key-tile order
        qg = np.arange(half * TOWN, (half + 1) * TOWN)
        kg = np.arange(LE)
        m["causal"] = np.ascontiguousarray(
            (kg[:, None] <= qg[None, :]).astype(BF16)
        ).reshape(NKT, P, TOWN)
        in_maps.append(m)
    return in_maps


def assemble(results):
    enc = np.zeros((B, LE, D), np.float32)
    dec = np.zeros((B, LD, D), np.float32)
    for c, r in enumerate(results):
        b, half = c // 2, c % 2
        sl = slice(half * TOWN, (half + 1) * TOWN)
        enc[b, sl, :] = r["enc_part"].reshape(D, TOWN).T
        dec[b, sl, :] = r["dec_part"].reshape(D, TOWN).T
    return enc, dec


def kernel(**inputs):
    from concourse import bass_utils

    if "nc" not in _CACHE:
        _CACHE["nc"] = build()
    nc = _CACHE["nc"]
    in_maps = make_in_maps(inputs)
    res = bass_utils.run_bass_kernel_spmd(
        nc, in_maps, core_ids=list(range(NCORE))
    )
    return assemble(res.results)


# revision 42
# speedup vs baseline: 1.0836x; 1.0309x over previous
"""Trainium2 Bass kernel for nn_EncoderDecoderTransformer (sparse kNN encoder attention).

Sharding: data-parallel over batch (4 batches x 2 cores); each pair of cores
splits the sequence (512 tokens each). Per attention sub-layer the pair
AllGathers the LN output h (bf16, 512KB) right after layer norm; each core then
projects K/V for the FULL sequence locally (PE has headroom), so the collective
is off the critical path (Q/K/V-own projections overlap the flight).
Cross-attention K/V are projected per decoder layer from a one-time AllGather
of enc_out - no per-layer cross collectives.

Layouts (per core):
  - Activations feature-major: x^T stored as 4 tiles (128 dims, 512 own tokens).
  - Q^T feature-major (head h lives in rows [64*(h%2):...] of ptile h//2).
  - K^T feature-major full-seq: 4 tiles [128, 1024] (columns = absolute token).
  - V token-major full-seq: 8 tiles (128 tokens, 8 heads, 65) with a constant-1
    column per head so the AV matmul also produces the softmax denominator.
  - Scores transposed: S^T = K^T.T @ Q^T, with BOTH heads of a ptile batched
    into one [128, 1024] PSUM tile so the Exp activation runs at N=1024
    (amortizes the ~352-cycle ACT overhead). kNN/causal masking is a 0/1 bf16
    multiply with column-duplicated [128,1024] mask tiles.
  - Decoder self-attention skips fully-masked key tiles (half-0 cores do 4 of
    8 kt) and skips the mask multiply on fully-allowed tiles (half-1, kt 0-3).
  - kNN mask: s'_qk = 2 x_q.x_k - |x_k|^2 orders like -distance; rank-17
    threshold via DVE max8/match_replace. Same math as the verified baseline.
"""

import os
import numpy as np
import ml_dtypes

BF16 = ml_dtypes.bfloat16

D, F, H, NE, ND, KNN = 512, 2048, 8, 4, 4, 16
B, LE, LD = 4, 1024, 1024
DH = D // H
NCORE = 8
P = 128
TOWN = 512          # tokens owned per core
T2 = 2 * TOWN       # batched free dim (two heads side by side)
NDT = D // P        # 4 feature tiles
NKT = LE // P       # 8 key tiles
NEG = -1e30
EPS = 1e-5
PAIRS = [[0, 1], [2, 3], [4, 5], [6, 7]]

_CACHE = {}


def build(n_enc=NE, n_dec=ND):
    from contextlib import ExitStack

    import concourse.bacc as bacc
    import concourse.tile as tile
    import concourse.mybir as mybir

    f32 = mybir.dt.float32
    bf16 = mybir.dt.bfloat16
    AF = mybir.ActivationFunctionType
    OP = mybir.AluOpType

    nc = bacc.Bacc("TRN2", target_bir_lowering=False, debug=False, num_devices=NCORE)

    # ---- I/O ----
    def din(name, shape, dt=f32):
        return nc.dram_tensor(name, shape, dt, kind="ExternalInput")

    x0T = din("x0T", [NDT, P, TOWN])
    y0T = din("y0T", [NDT, P, TOWN])
    xq2_d = din("xq2", [TOWN, 3])       # 2*xyz for own tokens
    xrow_d = din("xrow", [4, LE])        # [xyz, |xyz|^2] all tokens, transposed
    boscol_d = din("boscol", [TOWN, 1])  # threshold override col (NEG at BOS q)
    eye_in = din("eye", [P, P])
    causal_in = din("causal", [NKT, P, TOWN], bf16)  # per-core causal kt tiles

    ew_qkv = din("ew_qkv", [NE, D, 3 * D], bf16)
    ew_out = din("ew_out", [NE, D, D], bf16)
    ew_f1 = din("ew_f1", [NE, D, F], bf16)
    ew_f2 = din("ew_f2", [NE, F, D], bf16)
    eb_qkv = din("eb_qkv", [NE, 3 * D, 1])
    eb_out = din("eb_out", [NE, D, 1])
    eb_f1 = din("eb_f1", [NE, F, 1])
    eb_f2 = din("eb_f2", [NE, D, 1])

    dw_saqkv = din("dw_saqkv", [ND, D, 3 * D], bf16)
    db_saqkv = din("db_saqkv", [ND, 3 * D, 1])
    dw_saout = din("dw_saout", [ND, D, D], bf16)
    db_saout = din("db_saout", [ND, D, 1])
    dw_caqkv = din("dw_caqkv", [ND, D, 3 * D], bf16)
    db_caqkv = din("db_caqkv", [ND, 3 * D, 1])
    dw_caout = din("dw_caout", [ND, D, D], bf16)
    db_caout = din("db_caout", [ND, D, 1])
    dw_f1 = din("dw_f1", [ND, D, F], bf16)
    db_f1 = din("db_f1", [ND, F, 1])
    dw_f2 = din("dw_f2", [ND, F, D], bf16)
    db_f2 = din("db_f2", [ND, D, 1])
    eb_qkv_bf = din("eb_qkv_bf", [NE, 3 * D, 1], bf16)
    db_saqkv_bf = din("db_saqkv_bf", [ND, 3 * D, 1], bf16)
    db_caqkv_bf = din("db_caqkv_bf", [ND, 3 * D, 1], bf16)

    enc_part = nc.dram_tensor("enc_part", [NDT, P, TOWN], f32, kind="ExternalOutput")
    dec_part = nc.dram_tensor("dec_part", [NDT, P, TOWN], f32, kind="ExternalOutput")

    with tile.TileContext(nc) as tc, ExitStack() as ctx:
        ep = ctx.enter_context

        pc = ep(tc.tile_pool(name="pc", bufs=1))
        p_allow = ep(tc.tile_pool(name="p_allow", bufs=8))
        p_causal = ep(tc.tile_pool(name="p_causal", bufs=8))
        ps_s = ep(tc.tile_pool(name="ps_s", bufs=2, space="PSUM"))
        ps_o = ep(tc.tile_pool(name="ps_o", bufs=2, space="PSUM"))
        ps_mm = ep(tc.tile_pool(name="ps_mm", bufs=2, space="PSUM"))
        p_dram = ep(tc.tile_pool(name="p_dram", bufs=2, space="DRAM"))

        # ---- constants ----
        ones_col_bf = pc.tile([P, 1], bf16)
        nc.vector.memset(ones_col_bf, 1.0)
        ones_row = pc.tile([1, P], f32)
        nc.vector.memset(ones_row, 1.0)
        ones_row_bf = pc.tile([1, P], bf16)
        nc.vector.memset(ones_row_bf, 1.0)
        eps_sb = pc.tile([1, 1], f32)
        nc.vector.memset(eps_sb, EPS)
        epsD_sb = pc.tile([1, 1], f32)
        nc.vector.memset(epsD_sb, float(D) * D * EPS)
        lnD_sb = pc.tile([1, 1], f32)
        nc.vector.memset(lnD_sb, float(np.log(D)))
        dummy_w = pc.tile([P, P], bf16)
        nc.vector.memset(dummy_w, 0.0)
        dummy_x = pc.tile([P, TOWN], bf16)
        nc.vector.memset(dummy_x, 0.0)

        eye_f32 = pc.tile([P, P], f32)
        nc.sync.dma_start(out=eye_f32, in_=eye_in[:, :])

        causal_sb = []
        for i in range(NKT):
            t = p_causal.tile([P, TOWN], bf16, tag="causal")
            nc.sync.dma_start(out=t, in_=causal_in[i])
            causal_sb.append(t)

        def pe_warm(n):
            psD = ps_mm.tile([P, TOWN], f32, tag="mm")
            for _ in range(n):
                nc.tensor.matmul(psD, dummy_w, dummy_x, start=True, stop=True)

        def build_mask():
            # Exact-fp32 kNN scores s3 = 2 x_q.x_k - |x_k|^2 (orders like
            # -distance), computed query-major once; the rank-17 value (16 NN +
            # self) is the inclusion threshold. allow = (s3 >= thr) compares the
            # SAME fp32 values the threshold came from, then the 0/1 bf16 mask
            # is moved to key-major via exact PE transposes.
            import concourse.bass as cbass

            def bcast_rows(dram_row_ap, pool, n_free, tag):
                t = pool.tile([P, n_free], f32, tag=tag)
                src_ap = cbass.AP(
                    tensor=dram_row_ap.tensor, offset=dram_row_ap.offset,
                    ap=[[0, P]] + list(dram_row_ap.ap),
                )
                nc.sync.dma_start(out=t, in_=src_ap)
                return t

            allow_sb = []
            for kt in range(NKT):
                t = p_allow.tile([P, T2], bf16, tag="allow", name=f"allow{kt}")
                allow_sb.append(t)
            with tc.tile_pool(name="p_mask", bufs=3) as p_mask, \
                 tc.tile_pool(name="p_mbc", bufs=1) as p_mbc, \
                 tc.tile_pool(name="p_m8", bufs=8) as p_m8, \
                 tc.tile_pool(name="p_alq", bufs=2) as p_alq:
                bcx = []
                for c in range(4):
                    t = bcast_rows(xrow_d[c], p_mbc, LE, tag=f"bcx{c}")
                    bcx.append(t)
                for qt in range(4):
                    xqc = p_m8.tile([P, 3], f32, tag="xqc")
                    nc.sync.dma_start(out=xqc, in_=xq2_d[qt * P:(qt + 1) * P, :])
                    bosc = p_m8.tile([P, 1], f32, tag="bosc")
                    nc.sync.dma_start(out=bosc, in_=boscol_d[qt * P:(qt + 1) * P, :])
                    s0 = p_mask.tile([P, LE], f32, tag="s")
                    nc.vector.tensor_scalar(s0, bcx[0], xqc[:, 0:1], None, op0=OP.mult)
                    s1 = p_mask.tile([P, LE], f32, tag="s")
                    nc.vector.scalar_tensor_tensor(s1, bcx[1], xqc[:, 1:2], s0, OP.mult, OP.add)
                    s2 = p_mask.tile([P, LE], f32, tag="s")
                    nc.vector.scalar_tensor_tensor(s2, bcx[2], xqc[:, 2:3], s1, OP.mult, OP.add)
                    s3 = p_mask.tile([P, LE], f32, tag="s")
                    nc.vector.tensor_tensor(s3, s2, bcx[3], OP.subtract)
                    psW = ps_s.tile([P, T2], f32, tag="pss")
                    nc.tensor.matmul(psW[:, 0:TOWN], s3[:, 0:P], bcx[0][:, 0:TOWN],
                                     start=True, stop=True)
                    m8 = p_m8.tile([P, 8], f32, tag="m8")
                    nc.vector.max(m8, s3)
                    s4 = p_mask.tile([P, LE], f32, tag="s")
                    nc.vector.match_replace(s4, m8, s3, NEG)
                    m8b = p_m8.tile([P, 8], f32, tag="m8")
                    nc.vector.max(m8b, s4)
                    s5 = p_mask.tile([P, LE], f32, tag="s")
                    nc.vector.match_replace(s5, m8b, s4, NEG)
                    nc.tensor.matmul(psW[:, TOWN:T2], s5[:, 0:P], bcx[0][:, 0:TOWN],
                                     start=True, stop=True)
                    m8c = p_m8.tile([P, 8], f32, tag="m8")
                    nc.vector.max(m8c, s5)
                    thr = p_m8.tile([P, 1], f32, tag="thr")
                    nc.vector.tensor_tensor(thr, m8c[:, 0:1], bosc, OP.min)
                    al_q = p_alq.tile([P, LE], f32, tag="alq")
                    nc.vector.tensor_scalar(al_q, s3, thr, None, op0=OP.is_ge)
                    for kt in range(NKT):
                        psT = ps_mm.tile([P, P], f32, tag="mm")
                        nc.tensor.transpose(psT, al_q[:, kt * P:(kt + 1) * P], eye_f32)
                        nc.vector.tensor_copy(
                            allow_sb[kt][:, qt * P:(qt + 1) * P], psT
                        )
                for kt in range(NKT):
                    nc.vector.tensor_copy(allow_sb[kt][:, TOWN:T2], allow_sb[kt][:, 0:TOWN])
                nc.vector.memset(allow_sb[0][0:1, :], 1.0)  # BOS key allowed for all q
            return allow_sb

        pe_warm(12)
        allow_sb = build_mask()

        # ================= helpers =================
        def load_w(pool, dram_ap, kchunks, cols, tag):
            t = pool.tile([P, kchunks, cols], bf16, tag=tag)
            nc.sync.dma_start(
                out=t, in_=dram_ap.rearrange("(kc p) m -> p kc m", p=P)
            )
            return t

        def ps_alt(i):
            if i % 2 == 0:
                return ps_mm.tile([P, TOWN], f32, tag="mm", name=f"psa{i}")
            return ps_s.tile([P, TOWN], f32, tag="pss", name=f"psb{i}")

        def layer_norm(xs, out_dt, out_pool, out_tag):
            sqs = []
            xbs = []
            for dt in range(NDT):
                sq = p_lnsq.tile([P, TOWN], bf16, tag="lnsq")
                nc.scalar.activation(sq, xs[dt], AF.Square)
                sqs.append(sq)
                xb = p_lnsq.tile([P, TOWN], bf16, tag="lnxb")
                nc.vector.tensor_copy(xb, xs[dt])
                xbs.append(xb)
            ps_mean = ps_mm.tile([1, TOWN], f32, tag="mm")
            for dt in range(NDT):
                nc.tensor.matmul(ps_mean, ones_col_bf, xbs[dt], start=dt == 0, stop=dt == 3)
            ps_sq = ps_mm.tile([1, TOWN], f32, tag="mm")
            for dt in range(NDT):
                nc.tensor.matmul(ps_sq, ones_col_bf, sqs[dt], start=dt == 0, stop=dt == 3)
            # v' = D*sum(x^2) - sum(x)^2 = D^2*var; rstd = exp(-.5 ln(v'+D^2 eps) + ln D)
            psW = ps_s.tile([P, T2], f32, tag="pss")
            mu = p_small.tile([1, TOWN], f32, tag="sm")
            nc.vector.tensor_single_scalar(mu, ps_mean, 1.0 / D, OP.mult)
            s1sq = p_small.tile([1, TOWN], f32, tag="sm")
            nc.scalar.activation(s1sq, ps_mean, AF.Square)
            nc.tensor.matmul(psW[:, 0:TOWN], ones_row, s1sq, start=True, stop=True)
            var = p_small.tile([1, TOWN], f32, tag="sm")
            nc.vector.scalar_tensor_tensor(var, ps_sq, float(D), s1sq, OP.mult, OP.subtract)
            nc.tensor.matmul(psW[:, 0:TOWN], ones_row, var, start=True, stop=True)
            lnv = p_small.tile([1, TOWN], f32, tag="sm")
            nc.scalar.activation(lnv, var, AF.Ln, bias=epsD_sb)
            nc.tensor.matmul(psW[:, 0:TOWN], ones_row, lnv, start=True, stop=True)
            rstd = p_small.tile([1, TOWN], f32, tag="sm")
            nc.scalar.activation(rstd, lnv, AF.Exp, scale=-0.5, bias=lnD_sb)
            nc.tensor.matmul(psW[:, 0:TOWN], ones_row, rstd, start=True, stop=True)
            cro = p_small.tile([1, TOWN], f32, tag="sm")
            nc.vector.scalar_tensor_tensor(cro, mu, -1.0, rstd, OP.mult, OP.mult)
            ps_a = ps_mm.tile([P, TOWN], f32, tag="mm")
            nc.tensor.matmul(ps_a, ones_row, rstd, start=True, stop=True)
            ps_c = ps_mm.tile([P, TOWN], f32, tag="mm")
            nc.tensor.matmul(ps_c, ones_row, cro, start=True, stop=True)
            a_sb = p_lnac.tile([P, TOWN], f32, tag="lna")
            nc.vector.tensor_copy(a_sb, ps_a)
            c_sb = p_lnac.tile([P, TOWN], f32, tag="lnc")
            nc.vector.tensor_copy(c_sb, ps_c)
            hs = []
            for dt in range(NDT):
                h = out_pool.tile([P, TOWN], out_dt, tag=out_tag)
                nc.vector.tensor_tensor(h, xs[dt], a_sb, OP.mult)
                nc.vector.tensor_tensor(h, h, c_sb, OP.add)
                hs.append(h)
            return hs

        def proj_fm(w_sb, col_off, n_m, rhs, bias_ap, out_pool, out_tag, out_dt=bf16):
            """Feature-major projection; per-partition bias applied on eviction."""
            outs = []
            nk = len(rhs)
            for m in range(n_m):
                ps = ps_alt(m)
                for kc in range(nk):
                    nc.tensor.matmul(
                        ps, w_sb[:, kc, col_off + m * P:col_off + (m + 1) * P],
                        rhs[kc], start=kc == 0, stop=kc == nk - 1,
                    )
                bcol = p_bias.tile([P, 1], f32, tag="bcol")
                nc.sync.dma_start(out=bcol, in_=bias_ap[col_off + m * P:col_off + (m + 1) * P, :])
                o = out_pool.tile([P, TOWN], out_dt, tag=out_tag)
                nc.vector.tensor_scalar(o, ps, bcol, None, op0=OP.add)
                outs.append(o)
            return outs

        def kv_project(wk_sb, wv_sb, hs_half, hh, bias_ap, bias_bf_ap, Ks, Vs):
            """Project K (feature-major, into Ks column half hh) and V (token-
            major tiles Vs[4*hh + tt]) for the 512 tokens of half hh.
            bias_ap/bias_bf_ap hold [3D] packed qkv bias; K at +D, V at +2D."""
            csl = slice(hh * TOWN, (hh + 1) * TOWN)
            for m in range(NDT):
                ps = ps_alt(m)
                for kc in range(NDT):
                    nc.tensor.matmul(
                        ps, wk_sb[:, kc, m * P:(m + 1) * P],
                        hs_half[kc], start=kc == 0, stop=kc == NDT - 1,
                    )
                bcol = p_bias.tile([P, 1], f32, tag="bcol")
                nc.sync.dma_start(out=bcol, in_=bias_ap[D + m * P:D + (m + 1) * P, :])
                nc.scalar.activation(Ks[m][:, csl], ps, AF.Identity, bias=bcol)
            brow512 = p_bias.tile([1, TOWN], bf16, tag="brow512")
            nc.sync.dma_start(
                out=brow512,
                in_=bias_bf_ap[2 * D:3 * D, :].rearrange("a b -> b a"),
            )
            for tt in range(4):
                ps = ps_alt(tt)
                for kc in range(NDT):
                    nc.tensor.matmul(
                        ps, hs_half[kc][:, tt * P:(tt + 1) * P],
                        wv_sb[:, kc, 0:D],
                        start=kc == 0, stop=False,
                    )
                nc.tensor.matmul(ps, ones_row_bf, brow512, start=False, stop=True)
                vt = Vs[4 * hh + tt]
                nc.vector.tensor_copy(
                    vt[:, :, 0:64],
                    ps.rearrange("p (h d) -> p h d", h=H),
                )

        def alloc_vs():
            Vs = []
            for tt in range(NKT):
                vt = p_v.tile([P, H, 65], bf16, tag="vsb")
                nc.vector.memset(vt[:, :, 64:65], 1.0)
                Vs.append(vt)
            return Vs

        def attention(Qs, Ks, Vs, kt_order, masks, uniq):
            """masks: dict kt -> ("dup", [P,T2] tile) | ("half", [P,TOWN] tile).
            S matmuls run two kt ahead of AV (PE never stalls on exp/mask);
            the psO eviction chain of head-pair hp is emitted after the next
            head-pair's first S matmuls so its rec-wait overlaps real PE work."""
            OTs = []
            nkt = len(kt_order)
            pending = []

            def emit_evict(psO, ot):
                for j in range(2):
                    den = p_small.tile([1, TOWN], f32, tag="sm", name=f"den{j}")
                    nc.vector.tensor_copy(den, psO[j][64:65, :])
                    rec = p_small.tile([1, TOWN], f32, tag="sm", name=f"rec{j}")
                    nc.vector.reciprocal_approx_fast(rec, den)
                    psB = ps_mm.tile([64, TOWN], f32, tag="mm", name=f"psB{j}")
                    nc.tensor.matmul(psB, ones_row[:, 0:64], rec, start=True, stop=True)
                    bc = p_bc.tile([64, TOWN], f32, tag="bc", name=f"bc{j}")
                    nc.vector.tensor_copy(bc, psB)
                    nc.vector.tensor_tensor(ot[j * 64:(j + 1) * 64, :], psO[j][0:64, :], bc, OP.mult)

            for hp in range(4):
                psO = []
                for _j in range(2):
                    psO_t = ps_o.tile([65, TOWN], f32, tag="pso")
                    psO.append(psO_t)
                e2s = {}

                def emit_S(idx, hp=hp, e2s=e2s):
                    kt = kt_order[idx]
                    psS = ps_s.tile([P, T2], f32, tag="pss")
                    for j in range(2):
                        nc.tensor.matmul(
                            psS[:, j * TOWN:(j + 1) * TOWN],
                            Ks[hp][j * 64:(j + 1) * 64, kt * P:(kt + 1) * P],
                            Qs[hp][j * 64:(j + 1) * 64, :],
                            start=True, stop=True,
                        )
                    e = p_e.tile([P, T2], bf16, tag="e")
                    nc.scalar.activation(e, psS, AF.Exp, scale=0.125)
                    m = masks.get(kt)
                    if m is not None:
                        kind, mt = m
                        e2 = p_e.tile([P, T2], bf16, tag="e")
                        if kind == "dup":
                            nc.vector.tensor_tensor(e2, e, mt, OP.mult)
                        else:
                            for j in range(2):
                                jsl = slice(j * TOWN, (j + 1) * TOWN)
                                nc.vector.tensor_tensor(e2[:, jsl], e[:, jsl], mt, OP.mult)
                    else:
                        e2 = e
                    e2s[idx] = e2

                def emit_AV(idx, hp=hp, e2s=e2s, psO=psO):
                    kt = kt_order[idx]
                    e2 = e2s.pop(idx)
                    for j in range(2):
                        nc.tensor.matmul(
                            psO[j], Vs[kt][:, 2 * hp + j, :],
                            e2[:, j * TOWN:(j + 1) * TOWN],
                            start=idx == 0, stop=idx == nkt - 1,
                        )

                emit_S(0)
                if nkt > 1:
                    emit_S(1)
                if pending:
                    emit_evict(*pending.pop())
                for idx in range(nkt):
                    emit_AV(idx)
                    if idx + 2 < nkt:
                        emit_S(idx + 2)
                ot = p_ot.tile([P, TOWN], bf16, tag="ot")
                pending.append((psO, ot))
                OTs.append(ot)
            emit_evict(*pending.pop())
            return OTs

        def proj_residual(w_sb, col_off, n_k, rhs, bias_ap, xs):
            nxs = []
            for m in range(NDT):
                ps = ps_alt(m)
                for kc in range(n_k):
                    nc.tensor.matmul(
                        ps, w_sb[:, kc, col_off + m * P:col_off + (m + 1) * P],
                        rhs[kc], start=kc == 0, stop=kc == n_k - 1,
                    )
                bcol = p_bias.tile([P, 1], f32, tag="bcol")
                nc.sync.dma_start(out=bcol, in_=bias_ap[m * P:(m + 1) * P, :])
                nx = p_x.tile([P, TOWN], f32, tag="x")
                nc.vector.scalar_tensor_tensor(nx, ps, bcol, xs[m], OP.add, OP.add)
                nxs.append(nx)
            return nxs

        def ffn(w1_ap, w2_ap, b1_ap, b2_ap, hs, xs):
            gs = []
            for mp in range(F // P // 2):
                w1m = p_w1.tile([P, NDT, 2 * P], bf16, tag="wf1")
                nc.sync.dma_start(
                    out=w1m,
                    in_=w1_ap[:, mp * 2 * P:(mp + 1) * 2 * P].rearrange("(kc p) m -> p kc m", p=P),
                )
                for mi in range(2):
                    m = 2 * mp + mi
                    ps = ps_alt(m)
                    for kc in range(NDT):
                        nc.tensor.matmul(
                            ps, w1m[:, kc, mi * P:(mi + 1) * P], hs[kc],
                            start=kc == 0, stop=kc == NDT - 1,
                        )
                    bcol = p_bias.tile([P, 1], f32, tag="bcol")
                    nc.sync.dma_start(out=bcol, in_=b1_ap[m * P:(m + 1) * P, :])
                    g = p_g.tile([P, TOWN], bf16, tag="g")
                    nc.scalar.activation(g, ps, AF.Gelu, bias=bcol)
                    gs.append(g)
            nxs = []
            for m in range(NDT):
                w2m = p_w2.tile([P, F // P, P], bf16, tag="wf2")
                nc.sync.dma_start(
                    out=w2m,
                    in_=w2_ap[:, m * P:(m + 1) * P].rearrange("(kc p) c -> p kc c", p=P),
                )
                ps2 = ps_alt(m)
                for kc in range(F // P):
                    nc.tensor.matmul(
                        ps2, w2m[:, kc, :], gs[kc],
                        start=kc == 0, stop=kc == F // P - 1,
                    )
                bcol = p_bias.tile([P, 1], f32, tag="bcol")
                nc.sync.dma_start(out=bcol, in_=b2_ap[m * P:(m + 1) * P, :])
                nx = p_x.tile([P, TOWN], f32, tag="x")
                nc.vector.scalar_tensor_tensor(nx, ps2, bcol, xs[m], OP.add, OP.add)
                nxs.append(nx)
            return nxs

        def ag_h(hs, uniq):
            """DMA h tiles to a DRAM bounce and AllGather across the pair.
            Returns the gathered [2, NDT, P, TOWN] DRAM tile (index = half)."""
            hbin = p_dram.tile([NDT, P, TOWN], bf16, tag=f"hbin{uniq}")
            for dt in range(NDT):
                nc.sync.dma_start(out=hbin[dt], in_=hs[dt])
            hbout = p_dram.tile([2, NDT, P, TOWN], bf16, tag=f"hbout{uniq}")
            nc.gpsimd.collective_compute(
                "AllGather", OP.bypass, replica_groups=PAIRS,
                ins=[hbin[:].opt()], outs=[hbout[:].opt()],
            )
            return hbout

        def load_h_halves(hbout):
            halves = []
            for hh in range(2):
                tiles = []
                for dt in range(NDT):
                    t = p_h.tile([P, TOWN], bf16, tag="hall")
                    nc.sync.dma_start(out=t, in_=hbout[hh, dt])
                    # keep the HAM clock warm through the AllGather wait: a
                    # dummy matmul pinned on each arriving h tile
                    psW = ps_s.tile([P, T2], f32, tag="pss", name=f"pwh{hh}{dt}")
                    nc.tensor.matmul(psW[:, 0:TOWN], dummy_w, t, start=True, stop=True)
                    tiles.append(t)
                halves.append(tiles)
            return halves

        p_x = ep(tc.tile_pool(name="p_x", bufs=5))
        p_h = ep(tc.tile_pool(name="p_h", bufs=12))
        p_q = ep(tc.tile_pool(name="p_q", bufs=5))
        p_kv = ep(tc.tile_pool(name="p_kv", bufs=8))
        p_v = ep(tc.tile_pool(name="p_v", bufs=9))
        p_ot = ep(tc.tile_pool(name="p_ot", bufs=4))
        p_e = ep(tc.tile_pool(name="p_e", bufs=4))
        p_g = ep(tc.tile_pool(name="p_g", bufs=16))
        p_lnsq = ep(tc.tile_pool(name="p_lnsq", bufs=4))
        p_lnac = ep(tc.tile_pool(name="p_lnac", bufs=2))
        p_bc = ep(tc.tile_pool(name="p_bc", bufs=1))
        p_small = ep(tc.tile_pool(name="p_small", bufs=6))
        p_bias = ep(tc.tile_pool(name="p_bias", bufs=4))
        p_eo = ep(tc.tile_pool(name="p_eo", bufs=2))
        p_eoball = ep(tc.tile_pool(name="p_eoball", bufs=4))
        p_w1 = ep(tc.tile_pool(name="p_w1", bufs=2))
        p_w2 = ep(tc.tile_pool(name="p_w2", bufs=2))
        p_w = ep(tc.tile_pool(name="p_w", bufs=3))

        all_kt = list(range(NKT))
        sa_masks = {kt: ("half", causal_sb[kt]) for kt in range(NKT)}
        enc_masks = {kt: ("dup", allow_sb[kt]) for kt in range(NKT)}

        # ================= encoder =================
        xs = []
        for dt in range(NDT):
            x = p_x.tile([P, TOWN], f32, tag="x")
            nc.sync.dma_start(out=x, in_=x0T[dt])
            xs.append(x)

        for l in range(n_enc):
            wq = load_w(p_w, ew_qkv[l][:, 0:D], NDT, D, "w")
            wk = load_w(p_w, ew_qkv[l][:, D:2 * D], NDT, D, "w")
            wv = load_w(p_w, ew_qkv[l][:, 2 * D:3 * D], NDT, D, "w")
            wout = load_w(p_w, ew_out[l], NDT, D, "w")

            hs = layer_norm(xs, bf16, p_h, "h")
            hbout = ag_h(hs, f"e{l}")
            Qs = proj_fm(wq, 0, 4, hs, eb_qkv[l], p_q, "q")
            Ks = [p_kv.tile([P, LE], bf16, tag="ksb", name=f"ks{m}") for m in range(NDT)]
            Vs = alloc_vs()
            hhalves = load_h_halves(hbout)
            for hh in range(2):
                kv_project(wk, wv, hhalves[hh], hh, eb_qkv[l], eb_qkv_bf[l], Ks, Vs)
            OTs = attention(Qs, Ks, Vs, all_kt, enc_masks, f"e{l}")
            xs = proj_residual(wout, 0, NDT, OTs, eb_out[l], xs)
            hs = layer_norm(xs, bf16, p_h, "h")
            xs = ffn(ew_f1[l], ew_f2[l], eb_f1[l], eb_f2[l], hs, xs)

        eof = layer_norm(xs, f32, p_eo, "eof")
        eob = []
        for dt in range(NDT):
            nc.sync.dma_start(out=enc_part[dt], in_=eof[dt])
            t = p_h.tile([P, TOWN], bf16, tag="eob")
            nc.vector.tensor_copy(t, eof[dt])
            eob.append(t)
        ebout = ag_h(eob, "eo")
        eob_all = []
        for dt in range(NDT):
            t = p_eoball.tile([P, LE], bf16, tag="eoball")
            for hh in range(2):
                nc.sync.dma_start(
                    out=t[:, hh * TOWN:(hh + 1) * TOWN], in_=ebout[hh, dt]
                )
            eob_all.append(t)

        # ================= decoder =================
        ys = []
        for dt in range(NDT):
            y = p_x.tile([P, TOWN], f32, tag="x")
            nc.sync.dma_start(out=y, in_=y0T[dt])
            ys.append(y)

        def ca_k_project(l, wkv):
            caK = [p_kv.tile([P, LE], bf16, tag="ksb", name=f"cak{m}") for m in range(NDT)]
            for m in range(NDT):
                ps = ps_mm.tile([P, TOWN], f32, tag="mm")
                for kc in range(NDT):
                    nc.tensor.matmul(
                        ps, wkv[:, kc, m * P:(m + 1) * P],
                        eob_all[kc][:, 0:TOWN], start=kc == 0, stop=kc == NDT - 1,
                    )
                ps2 = ps_s.tile([P, TOWN], f32, tag="pss")
                for kc in range(NDT):
                    nc.tensor.matmul(
                        ps2, wkv[:, kc, m * P:(m + 1) * P],
                        eob_all[kc][:, TOWN:LE], start=kc == 0, stop=kc == NDT - 1,
                    )
                bcol = p_bias.tile([P, 1], f32, tag="bcol")
                nc.sync.dma_start(out=bcol, in_=db_caqkv[l][D + m * P:D + (m + 1) * P, :])
                nc.scalar.activation(caK[m][:, 0:TOWN], ps, AF.Identity, bias=bcol)
                nc.scalar.activation(caK[m][:, TOWN:LE], ps2, AF.Identity, bias=bcol)
            return caK

        def ca_v_project(l):
            wkv = load_w(p_w, dw_caqkv[l][:, 2 * D:3 * D], NDT, D, "w")
            brow512 = p_bias.tile([1, TOWN], bf16, tag="brow512")
            nc.sync.dma_start(
                out=brow512,
                in_=db_caqkv_bf[l][2 * D:3 * D, :].rearrange("a b -> b a"),
            )
            caV = alloc_vs()
            for tt in range(NKT):
                ps = ps_alt(tt)
                for kc in range(NDT):
                    nc.tensor.matmul(
                        ps, eob_all[kc][:, tt * P:(tt + 1) * P],
                        wkv[:, kc, 0:D],
                        start=kc == 0, stop=False,
                    )
                nc.tensor.matmul(ps, ones_row_bf, brow512, start=False, stop=True)
                nc.vector.tensor_copy(
                    caV[tt][:, :, 0:64],
                    ps.rearrange("p (h d) -> p h d", h=H),
                )
            return caV

        for l in range(n_dec):
            wq = load_w(p_w, dw_saqkv[l][:, 0:D], NDT, D, "w")
            wkv_ca = load_w(p_w, dw_caqkv[l][:, D:2 * D], NDT, D, "w")
            wk = load_w(p_w, dw_saqkv[l][:, D:2 * D], NDT, D, "w")
            wv = load_w(p_w, dw_saqkv[l][:, 2 * D:3 * D], NDT, D, "w")
            wout = load_w(p_w, dw_saout[l], NDT, D, "w")

            # self-attention (causal)
            hs = layer_norm(ys, bf16, p_h, "h")
            hbout = ag_h(hs, f"d{l}")
            Qs = proj_fm(wq, 0, 4, hs, db_saqkv[l], p_q, "q")
            # independent work to cover the AllGather flight:
            caK = ca_k_project(l, wkv_ca)
            Ks = [p_kv.tile([P, LE], bf16, tag="ksb", name=f"ks{m}") for m in range(NDT)]
            Vs = alloc_vs()
            hhalves = load_h_halves(hbout)
            for hh in range(2):
                kv_project(wk, wv, hhalves[hh], hh, db_saqkv[l], db_saqkv_bf[l], Ks, Vs)
            OTs = attention(Qs, Ks, Vs, all_kt, sa_masks, f"s{l}")
            ys = proj_residual(wout, 0, NDT, OTs, db_saout[l], ys)
            caV = ca_v_project(l)  # fills the LN2-chain PE bubble

            # cross-attention (no mask)
            wcaq = load_w(p_w, dw_caqkv[l][:, 0:D], NDT, D, "w")
            wcao = load_w(p_w, dw_caout[l], NDT, D, "w")
            hs = layer_norm(ys, bf16, p_h, "h")
            Qs = proj_fm(wcaq, 0, 4, hs, db_caqkv[l], p_q, "q")
            OTs = attention(Qs, caK, caV, list(range(NKT)), {}, f"c{l}")
            ys = proj_residual(wcao, 0, NDT, OTs, db_caout[l], ys)

            # ffn
            hs = layer_norm(ys, bf16, p_h, "h")
            ys = ffn(dw_f1[l], dw_f2[l], db_f1[l], db_f2[l], hs, ys)

        dof = layer_norm(ys, f32, p_eo, "eof")
        for dt in range(NDT):
            nc.sync.dma_start(out=dec_part[dt], in_=dof[dt])

    nc.compile()
    return nc


def make_in_maps(inputs):
    inp = {k: np.asarray(v) for k, v in inputs.items()}
    f32 = np.float32

    W = {
        "ew_qkv": np.ascontiguousarray(inp["e_qkv_w"].swapaxes(1, 2)).astype(BF16),
        "ew_out": np.ascontiguousarray(inp["e_out_w"].swapaxes(1, 2)).astype(BF16),
        "ew_f1": np.ascontiguousarray(inp["e_ff1_w"].swapaxes(1, 2)).astype(BF16),
        "ew_f2": np.ascontiguousarray(inp["e_ff2_w"].swapaxes(1, 2)).astype(BF16),
        "eb_qkv": inp["e_qkv_b"].astype(f32).reshape(NE, 3 * D, 1),
        "eb_out": inp["e_out_b"].astype(f32).reshape(NE, D, 1),
        "eb_f1": inp["e_ff1_b"].astype(f32).reshape(NE, F, 1),
        "eb_f2": inp["e_ff2_b"].astype(f32).reshape(NE, D, 1),
        "dw_saqkv": np.ascontiguousarray(inp["d_sa_qkv_w"].swapaxes(1, 2)).astype(BF16),
        "db_saqkv": inp["d_sa_qkv_b"].astype(f32).reshape(ND, 3 * D, 1),
        "dw_saout": np.ascontiguousarray(inp["d_sa_out_w"].swapaxes(1, 2)).astype(BF16),
        "db_saout": inp["d_sa_out_b"].astype(f32).reshape(ND, D, 1),
        "dw_caqkv": np.ascontiguousarray(inp["d_ca_qkv_w"].swapaxes(1, 2)).astype(BF16),
        "db_caqkv": inp["d_ca_qkv_b"].astype(f32).reshape(ND, 3 * D, 1),
        "dw_caout": np.ascontiguousarray(inp["d_ca_out_w"].swapaxes(1, 2)).astype(BF16),
        "db_caout": inp["d_ca_out_b"].astype(f32).reshape(ND, D, 1),
        "dw_f1": np.ascontiguousarray(inp["d_ff1_w"].swapaxes(1, 2)).astype(BF16),
        "db_f1": inp["d_ff1_b"].astype(f32).reshape(ND, F, 1),
        "dw_f2": np.ascontiguousarray(inp["d_ff2_w"].swapaxes(1, 2)).astype(BF16),
        "db_f2": inp["d_ff2_b"].astype(f32).reshape(ND, D, 1),
        "eb_qkv_bf": inp["e_qkv_b"].astype(BF16).reshape(NE, 3 * D, 1),
        "db_saqkv_bf": inp["d_sa_qkv_b"].astype(BF16).reshape(ND, 3 * D, 1),
        "db_caqkv_bf": inp["d_ca_qkv_b"].astype(BF16).reshape(ND, 3 * D, 1),
    }

    in_maps = []
    for c in range(NCORE):
        b, half = c // 2, c % 2
        sl = slice(half * TOWN, (half + 1) * TOWN)
        m = dict(W)
        xT = np.ascontiguousarray(inp["enc_in"][b].astype(f32).T[:, sl])
        m["x0T"] = xT.reshape(NDT, P, TOWN)
        yT = np.ascontiguousarray(inp["dec_in"][b].astype(f32).T[:, sl])
        m["y0T"] = yT.reshape(NDT, P, TOWN)
        xyz = inp["enc_xyz"][b].astype(f32)
        n2 = (xyz * xyz).sum(-1, dtype=f32).astype(f32)
        xq2 = (np.float32(2.0) * xyz[sl]).astype(f32)
        m["xq2"] = np.ascontiguousarray(xq2)
        xkn = np.concatenate([xyz, n2[:, None]], 1).astype(f32)
        m["xrow"] = np.ascontiguousarray(xkn.T)
        bos = np.full((TOWN, 1), 1e30, f32)
        if half == 0:
            bos[0, 0] = NEG
        m["boscol"] = bos
        m["eye"] = np.eye(P, dtype=np.float32)
        # causal tiles vs own queries, absolute key-tile order
        qg = np.arange(half * TOWN, (half + 1) * TOWN)
        kg = np.arange(LE)
        m["causal"] = np.ascontiguousarray(
            (kg[:, None] <= qg[None, :]).astype(BF16)
        ).reshape(NKT, P, TOWN)
        in_maps.append(m)
    return in_maps


def assemble(results):
    enc = np.zeros((B, LE, D), np.float32)
    dec = np.zeros((B, LD, D), np.float32)
    for c, r in enumerate(results):
        b, half = c // 2, c % 2
        sl = slice(half * TOWN, (half + 1) * TOWN)
        enc[b, sl, :] = r["enc_part"].reshape(D, TOWN).T
        dec[b, sl, :] = r["dec_part"].reshape(D, TOWN).T
    return enc, dec


def kernel(**inputs):
    from concourse import bass_utils

    if "nc" not in _CACHE:
        _CACHE["nc"] = build()
    nc = _CACHE["nc"]
    in_maps = make_in_maps(inputs)
    res = bass_utils.run_bass_kernel_spmd(
        nc, in_maps, core_ids=list(range(NCORE))
    )
    return assemble(res.results)


# revision 43
# speedup vs baseline: 1.0888x; 1.0048x over previous
"""Trainium2 Bass kernel for nn_EncoderDecoderTransformer (sparse kNN encoder attention).

Sharding: data-parallel over batch (4 batches x 2 cores); each pair of cores
splits the sequence (512 tokens each). Per attention sub-layer the pair
AllGathers the LN output h (bf16, 512KB) right after layer norm; each core then
projects K/V for the FULL sequence locally (PE has headroom), so the collective
is off the critical path (Q/K/V-own projections overlap the flight).
Cross-attention K/V are projected per decoder layer from a one-time AllGather
of enc_out - no per-layer cross collectives.

Layouts (per core):
  - Activations feature-major: x^T stored as 4 tiles (128 dims, 512 own tokens).
  - Q^T feature-major (head h lives in rows [64*(h%2):...] of ptile h//2).
  - K^T feature-major full-seq: 4 tiles [128, 1024] (columns = absolute token).
  - V token-major full-seq: 8 tiles (128 tokens, 8 heads, 65) with a constant-1
    column per head so the AV matmul also produces the softmax denominator.
  - Scores transposed: S^T = K^T.T @ Q^T, with BOTH heads of a ptile batched
    into one [128, 1024] PSUM tile so the Exp activation runs at N=1024
    (amortizes the ~352-cycle ACT overhead). kNN/causal masking is a 0/1 bf16
    multiply with column-duplicated [128,1024] mask tiles.
  - Decoder self-attention skips fully-masked key tiles (half-0 cores do 4 of
    8 kt) and skips the mask multiply on fully-allowed tiles (half-1, kt 0-3).
  - kNN mask: s'_qk = 2 x_q.x_k - |x_k|^2 orders like -distance; rank-17
    threshold via DVE max8/match_replace. Same math as the verified baseline.
"""

import os
import numpy as np
import ml_dtypes

BF16 = ml_dtypes.bfloat16

D, F, H, NE, ND, KNN = 512, 2048, 8, 4, 4, 16
B, LE, LD = 4, 1024, 1024
DH = D // H
NCORE = 8
P = 128
TOWN = 512          # tokens owned per core
T2 = 2 * TOWN       # batched free dim (two heads side by side)
NDT = D // P        # 4 feature tiles
NKT = LE // P       # 8 key tiles
NEG = -1e30
EPS = 1e-5
PAIRS = [[0, 1], [2, 3], [4, 5], [6, 7]]

_CACHE = {}


def build(n_enc=NE, n_dec=ND):
    from contextlib import ExitStack

    import concourse.bacc as bacc
    import concourse.tile as tile
    import concourse.mybir as mybir

    f32 = mybir.dt.float32
    bf16 = mybir.dt.bfloat16
    AF = mybir.ActivationFunctionType
    OP = mybir.AluOpType

    nc = bacc.Bacc("TRN2", target_bir_lowering=False, debug=False, num_devices=NCORE)

    # ---- I/O ----
    def din(name, shape, dt=f32):
        return nc.dram_tensor(name, shape, dt, kind="ExternalInput")

    x0T = din("x0T", [NDT, P, TOWN])
    y0T = din("y0T", [NDT, P, TOWN])
    xq2_d = din("xq2", [TOWN, 3])       # 2*xyz for own tokens
    xrow_d = din("xrow", [4, LE])        # [xyz, |xyz|^2] all tokens, transposed
    boscol_d = din("boscol", [TOWN, 1])  # threshold override col (NEG at BOS q)
    eye_in = din("eye", [P, P])
    causal_in = din("causal", [NKT, P, TOWN], bf16)  # per-core causal kt tiles

    ew_qkv = din("ew_qkv", [NE, D, 3 * D], bf16)
    ew_out = din("ew_out", [NE, D, D], bf16)
    ew_f1 = din("ew_f1", [NE, D, F], bf16)
    ew_f2 = din("ew_f2", [NE, F, D], bf16)
    eb_qkv = din("eb_qkv", [NE, 3 * D, 1])
    eb_out = din("eb_out", [NE, D, 1])
    eb_f1 = din("eb_f1", [NE, F, 1])
    eb_f2 = din("eb_f2", [NE, D, 1])

    dw_saqkv = din("dw_saqkv", [ND, D, 3 * D], bf16)
    db_saqkv = din("db_saqkv", [ND, 3 * D, 1])
    dw_saout = din("dw_saout", [ND, D, D], bf16)
    db_saout = din("db_saout", [ND, D, 1])
    dw_caqkv = din("dw_caqkv", [ND, D, 3 * D], bf16)
    db_caqkv = din("db_caqkv", [ND, 3 * D, 1])
    dw_caout = din("dw_caout", [ND, D, D], bf16)
    db_caout = din("db_caout", [ND, D, 1])
    dw_f1 = din("dw_f1", [ND, D, F], bf16)
    db_f1 = din("db_f1", [ND, F, 1])
    dw_f2 = din("dw_f2", [ND, F, D], bf16)
    db_f2 = din("db_f2", [ND, D, 1])
    ebp_d = din("ebp", [NE, P, 32])
    dbp_d = din("dbp", [ND, P, 44])
    eb_qkv_bf = din("eb_qkv_bf", [NE, 3 * D, 1], bf16)
    db_saqkv_bf = din("db_saqkv_bf", [ND, 3 * D, 1], bf16)
    db_caqkv_bf = din("db_caqkv_bf", [ND, 3 * D, 1], bf16)

    enc_part = nc.dram_tensor("enc_part", [NDT, P, TOWN], f32, kind="ExternalOutput")
    dec_part = nc.dram_tensor("dec_part", [NDT, P, TOWN], f32, kind="ExternalOutput")

    with tile.TileContext(nc) as tc, ExitStack() as ctx:
        ep = ctx.enter_context

        pc = ep(tc.tile_pool(name="pc", bufs=1))
        p_allow = ep(tc.tile_pool(name="p_allow", bufs=8))
        p_causal = ep(tc.tile_pool(name="p_causal", bufs=8))
        ps_s = ep(tc.tile_pool(name="ps_s", bufs=2, space="PSUM"))
        ps_o = ep(tc.tile_pool(name="ps_o", bufs=2, space="PSUM"))
        ps_mm = ep(tc.tile_pool(name="ps_mm", bufs=2, space="PSUM"))
        p_dram = ep(tc.tile_pool(name="p_dram", bufs=2, space="DRAM"))

        # ---- constants ----
        ones_col_bf = pc.tile([P, 1], bf16)
        nc.vector.memset(ones_col_bf, 1.0)
        ones_row = pc.tile([1, P], f32)
        nc.vector.memset(ones_row, 1.0)
        ones_row_bf = pc.tile([1, P], bf16)
        nc.vector.memset(ones_row_bf, 1.0)
        eps_sb = pc.tile([1, 1], f32)
        nc.vector.memset(eps_sb, EPS)
        epsD_sb = pc.tile([1, 1], f32)
        nc.vector.memset(epsD_sb, float(D) * D * EPS)
        lnD_sb = pc.tile([1, 1], f32)
        nc.vector.memset(lnD_sb, float(np.log(D)))
        dummy_w = pc.tile([P, P], bf16)
        nc.vector.memset(dummy_w, 0.0)
        dummy_x = pc.tile([P, TOWN], bf16)
        nc.vector.memset(dummy_x, 0.0)

        eye_f32 = pc.tile([P, P], f32)
        nc.sync.dma_start(out=eye_f32, in_=eye_in[:, :])

        causal_sb = []
        for i in range(NKT):
            t = p_causal.tile([P, TOWN], bf16, tag="causal")
            nc.sync.dma_start(out=t, in_=causal_in[i])
            causal_sb.append(t)

        def pe_warm(n):
            psD = ps_mm.tile([P, TOWN], f32, tag="mm")
            for _ in range(n):
                nc.tensor.matmul(psD, dummy_w, dummy_x, start=True, stop=True)

        def build_mask():
            # Exact-fp32 kNN scores s3 = 2 x_q.x_k - |x_k|^2 (orders like
            # -distance), computed query-major once; the rank-17 value (16 NN +
            # self) is the inclusion threshold. allow = (s3 >= thr) compares the
            # SAME fp32 values the threshold came from, then the 0/1 bf16 mask
            # is moved to key-major via exact PE transposes.
            import concourse.bass as cbass

            def bcast_rows(dram_row_ap, pool, n_free, tag):
                t = pool.tile([P, n_free], f32, tag=tag)
                src_ap = cbass.AP(
                    tensor=dram_row_ap.tensor, offset=dram_row_ap.offset,
                    ap=[[0, P]] + list(dram_row_ap.ap),
                )
                nc.sync.dma_start(out=t, in_=src_ap)
                return t

            allow_sb = []
            for kt in range(NKT):
                t = p_allow.tile([P, T2], bf16, tag="allow", name=f"allow{kt}")
                allow_sb.append(t)
            with tc.tile_pool(name="p_mask", bufs=3) as p_mask, \
                 tc.tile_pool(name="p_mbc", bufs=1) as p_mbc, \
                 tc.tile_pool(name="p_m8", bufs=8) as p_m8, \
                 tc.tile_pool(name="p_alq", bufs=2) as p_alq:
                bcx = []
                for c in range(4):
                    t = bcast_rows(xrow_d[c], p_mbc, LE, tag=f"bcx{c}")
                    bcx.append(t)
                for qt in range(4):
                    xqc = p_m8.tile([P, 3], f32, tag="xqc")
                    nc.sync.dma_start(out=xqc, in_=xq2_d[qt * P:(qt + 1) * P, :])
                    bosc = p_m8.tile([P, 1], f32, tag="bosc")
                    nc.sync.dma_start(out=bosc, in_=boscol_d[qt * P:(qt + 1) * P, :])
                    s0 = p_mask.tile([P, LE], f32, tag="s")
                    nc.vector.tensor_scalar(s0, bcx[0], xqc[:, 0:1], None, op0=OP.mult)
                    s1 = p_mask.tile([P, LE], f32, tag="s")
                    nc.vector.scalar_tensor_tensor(s1, bcx[1], xqc[:, 1:2], s0, OP.mult, OP.add)
                    s2 = p_mask.tile([P, LE], f32, tag="s")
                    nc.vector.scalar_tensor_tensor(s2, bcx[2], xqc[:, 2:3], s1, OP.mult, OP.add)
                    s3 = p_mask.tile([P, LE], f32, tag="s")
                    nc.vector.tensor_tensor(s3, s2, bcx[3], OP.subtract)
                    psW = ps_s.tile([P, T2], f32, tag="pss")
                    nc.tensor.matmul(psW[:, 0:TOWN], s3[:, 0:P], bcx[0][:, 0:TOWN],
                                     start=True, stop=True)
                    m8 = p_m8.tile([P, 8], f32, tag="m8")
                    nc.vector.max(m8, s3)
                    s4 = p_mask.tile([P, LE], f32, tag="s")
                    nc.vector.match_replace(s4, m8, s3, NEG)
                    m8b = p_m8.tile([P, 8], f32, tag="m8")
                    nc.vector.max(m8b, s4)
                    s5 = p_mask.tile([P, LE], f32, tag="s")
                    nc.vector.match_replace(s5, m8b, s4, NEG)
                    nc.tensor.matmul(psW[:, TOWN:T2], s5[:, 0:P], bcx[0][:, 0:TOWN],
                                     start=True, stop=True)
                    m8c = p_m8.tile([P, 8], f32, tag="m8")
                    nc.vector.max(m8c, s5)
                    thr = p_m8.tile([P, 1], f32, tag="thr")
                    nc.vector.tensor_tensor(thr, m8c[:, 0:1], bosc, OP.min)
                    al_q = p_alq.tile([P, LE], f32, tag="alq")
                    nc.vector.tensor_scalar(al_q, s3, thr, None, op0=OP.is_ge)
                    for kt in range(NKT):
                        psT = ps_mm.tile([P, P], f32, tag="mm")
                        nc.tensor.transpose(psT, al_q[:, kt * P:(kt + 1) * P], eye_f32)
                        nc.vector.tensor_copy(
                            allow_sb[kt][:, qt * P:(qt + 1) * P], psT
                        )
                for kt in range(NKT):
                    nc.vector.tensor_copy(allow_sb[kt][:, TOWN:T2], allow_sb[kt][:, 0:TOWN])
                nc.vector.memset(allow_sb[0][0:1, :], 1.0)  # BOS key allowed for all q
            return allow_sb

        pe_warm(12)
        allow_sb = build_mask()

        # ================= helpers =================
        def load_w(pool, dram_ap, kchunks, cols, tag):
            t = pool.tile([P, kchunks, cols], bf16, tag=tag)
            nc.sync.dma_start(
                out=t, in_=dram_ap.rearrange("(kc p) m -> p kc m", p=P)
            )
            return t

        def ps_alt(i):
            if i % 2 == 0:
                return ps_mm.tile([P, TOWN], f32, tag="mm", name=f"psa{i}")
            return ps_s.tile([P, TOWN], f32, tag="pss", name=f"psb{i}")

        def layer_norm(xs, out_dt, out_pool, out_tag):
            sqs = []
            xbs = []
            for dt in range(NDT):
                sq = p_lnsq.tile([P, TOWN], bf16, tag="lnsq")
                nc.scalar.activation(sq, xs[dt], AF.Square)
                sqs.append(sq)
                xb = p_lnsq.tile([P, TOWN], bf16, tag="lnxb")
                nc.vector.tensor_copy(xb, xs[dt])
                xbs.append(xb)
            ps_mean = ps_mm.tile([1, TOWN], f32, tag="mm")
            for dt in range(NDT):
                nc.tensor.matmul(ps_mean, ones_col_bf, xbs[dt], start=dt == 0, stop=dt == 3)
            ps_sq = ps_mm.tile([1, TOWN], f32, tag="mm")
            for dt in range(NDT):
                nc.tensor.matmul(ps_sq, ones_col_bf, sqs[dt], start=dt == 0, stop=dt == 3)
            # v' = D*sum(x^2) - sum(x)^2 = D^2*var; rstd = exp(-.5 ln(v'+D^2 eps) + ln D)
            psW = ps_s.tile([P, T2], f32, tag="pss")
            mu = p_small.tile([1, TOWN], f32, tag="sm")
            nc.vector.tensor_single_scalar(mu, ps_mean, 1.0 / D, OP.mult)
            s1sq = p_small.tile([1, TOWN], f32, tag="sm")
            nc.scalar.activation(s1sq, ps_mean, AF.Square)
            nc.tensor.matmul(psW[:, 0:TOWN], ones_row, s1sq, start=True, stop=True)
            var = p_small.tile([1, TOWN], f32, tag="sm")
            nc.vector.scalar_tensor_tensor(var, ps_sq, float(D), s1sq, OP.mult, OP.subtract)
            nc.tensor.matmul(psW[:, 0:TOWN], ones_row, var, start=True, stop=True)
            lnv = p_small.tile([1, TOWN], f32, tag="sm")
            nc.scalar.activation(lnv, var, AF.Ln, bias=epsD_sb)
            nc.tensor.matmul(psW[:, 0:TOWN], ones_row, lnv, start=True, stop=True)
            rstd = p_small.tile([1, TOWN], f32, tag="sm")
            nc.scalar.activation(rstd, lnv, AF.Exp, scale=-0.5, bias=lnD_sb)
            nc.tensor.matmul(psW[:, 0:TOWN], ones_row, rstd, start=True, stop=True)
            cro = p_small.tile([1, TOWN], f32, tag="sm")
            nc.vector.scalar_tensor_tensor(cro, mu, -1.0, rstd, OP.mult, OP.mult)
            ps_a = ps_mm.tile([P, TOWN], f32, tag="mm")
            nc.tensor.matmul(ps_a, ones_row, rstd, start=True, stop=True)
            ps_c = ps_mm.tile([P, TOWN], f32, tag="mm")
            nc.tensor.matmul(ps_c, ones_row, cro, start=True, stop=True)
            a_sb = p_lnac.tile([P, TOWN], f32, tag="lna")
            nc.vector.tensor_copy(a_sb, ps_a)
            c_sb = p_lnac.tile([P, TOWN], f32, tag="lnc")
            nc.vector.tensor_copy(c_sb, ps_c)
            hs = []
            for dt in range(NDT):
                h = out_pool.tile([P, TOWN], out_dt, tag=out_tag)
                nc.vector.tensor_tensor(h, xs[dt], a_sb, OP.mult)
                nc.vector.tensor_tensor(h, h, c_sb, OP.add)
                hs.append(h)
            return hs

        def proj_fm(w_sb, col_off, n_m, rhs, bp, bbase, out_pool, out_tag, out_dt=bf16):
            """Feature-major projection; per-partition bias applied on eviction."""
            outs = []
            nk = len(rhs)
            for m in range(n_m):
                ps = ps_alt(m)
                for kc in range(nk):
                    nc.tensor.matmul(
                        ps, w_sb[:, kc, col_off + m * P:col_off + (m + 1) * P],
                        rhs[kc], start=kc == 0, stop=kc == nk - 1,
                    )
                o = out_pool.tile([P, TOWN], out_dt, tag=out_tag)
                nc.vector.tensor_scalar(o, ps, bp[:, bbase + m:bbase + m + 1], None, op0=OP.add)
                outs.append(o)
            return outs

        def kv_project(wk_sb, wv_sb, hs_half, hh, bp, bbase, bias_bf_ap, Ks, Vs):
            """Project K (feature-major, into Ks column half hh) and V (token-
            major tiles Vs[4*hh + tt]) for the 512 tokens of half hh."""
            csl = slice(hh * TOWN, (hh + 1) * TOWN)
            for m in range(NDT):
                ps = ps_alt(m)
                for kc in range(NDT):
                    nc.tensor.matmul(
                        ps, wk_sb[:, kc, m * P:(m + 1) * P],
                        hs_half[kc], start=kc == 0, stop=kc == NDT - 1,
                    )
                nc.scalar.activation(Ks[m][:, csl], ps, AF.Identity,
                                     bias=bp[:, bbase + m:bbase + m + 1])
            brow512 = p_bias.tile([1, TOWN], bf16, tag="brow512")
            nc.sync.dma_start(
                out=brow512,
                in_=bias_bf_ap[2 * D:3 * D, :].rearrange("a b -> b a"),
            )
            for tt in range(4):
                ps = ps_alt(tt)
                for kc in range(NDT):
                    nc.tensor.matmul(
                        ps, hs_half[kc][:, tt * P:(tt + 1) * P],
                        wv_sb[:, kc, 0:D],
                        start=kc == 0, stop=False,
                    )
                nc.tensor.matmul(ps, ones_row_bf, brow512, start=False, stop=True)
                vt = Vs[4 * hh + tt]
                nc.vector.tensor_copy(
                    vt[:, :, 0:64],
                    ps.rearrange("p (h d) -> p h d", h=H),
                )

        def alloc_vs():
            Vs = []
            for tt in range(NKT):
                vt = p_v.tile([P, H, 65], bf16, tag="vsb")
                nc.vector.memset(vt[:, :, 64:65], 1.0)
                Vs.append(vt)
            return Vs

        def attention(Qs, Ks, Vs, kt_order, masks, uniq):
            """masks: dict kt -> ("dup", [P,T2] tile) | ("half", [P,TOWN] tile).
            S matmuls run two kt ahead of AV (PE never stalls on exp/mask);
            the psO eviction chain of head-pair hp is emitted after the next
            head-pair's first S matmuls so its rec-wait overlaps real PE work."""
            OTs = []
            nkt = len(kt_order)
            pending = []

            def emit_evict(psO, ot):
                for j in range(2):
                    den = p_small.tile([1, TOWN], f32, tag="sm", name=f"den{j}")
                    nc.vector.tensor_copy(den, psO[j][64:65, :])
                    rec = p_small.tile([1, TOWN], f32, tag="sm", name=f"rec{j}")
                    nc.vector.reciprocal_approx_fast(rec, den)
                    psB = ps_mm.tile([64, TOWN], f32, tag="mm", name=f"psB{j}")
                    nc.tensor.matmul(psB, ones_row[:, 0:64], rec, start=True, stop=True)
                    bc = p_bc.tile([64, TOWN], f32, tag="bc", name=f"bc{j}")
                    nc.vector.tensor_copy(bc, psB)
                    nc.vector.tensor_tensor(ot[j * 64:(j + 1) * 64, :], psO[j][0:64, :], bc, OP.mult)

            for hp in range(4):
                psO = []
                for _j in range(2):
                    psO_t = ps_o.tile([65, TOWN], f32, tag="pso")
                    psO.append(psO_t)
                e2s = {}

                def emit_S(idx, hp=hp, e2s=e2s):
                    kt = kt_order[idx]
                    psS = ps_s.tile([P, T2], f32, tag="pss")
                    for j in range(2):
                        nc.tensor.matmul(
                            psS[:, j * TOWN:(j + 1) * TOWN],
                            Ks[hp][j * 64:(j + 1) * 64, kt * P:(kt + 1) * P],
                            Qs[hp][j * 64:(j + 1) * 64, :],
                            start=True, stop=True,
                        )
                    e = p_e.tile([P, T2], bf16, tag="e")
                    nc.scalar.activation(e, psS, AF.Exp, scale=0.125)
                    m = masks.get(kt)
                    if m is not None:
                        kind, mt = m
                        e2 = p_e.tile([P, T2], bf16, tag="e")
                        if kind == "dup":
                            nc.vector.tensor_tensor(e2, e, mt, OP.mult)
                        else:
                            for j in range(2):
                                jsl = slice(j * TOWN, (j + 1) * TOWN)
                                nc.vector.tensor_tensor(e2[:, jsl], e[:, jsl], mt, OP.mult)
                    else:
                        e2 = e
                    e2s[idx] = e2

                def emit_AV(idx, hp=hp, e2s=e2s, psO=psO):
                    kt = kt_order[idx]
                    e2 = e2s.pop(idx)
                    for j in range(2):
                        nc.tensor.matmul(
                            psO[j], Vs[kt][:, 2 * hp + j, :],
                            e2[:, j * TOWN:(j + 1) * TOWN],
                            start=idx == 0, stop=idx == nkt - 1,
                        )

                emit_S(0)
                if nkt > 1:
                    emit_S(1)
                if pending:
                    emit_evict(*pending.pop())
                for idx in range(nkt):
                    emit_AV(idx)
                    if idx + 2 < nkt:
                        emit_S(idx + 2)
                ot = p_ot.tile([P, TOWN], bf16, tag="ot")
                pending.append((psO, ot))
                OTs.append(ot)
            emit_evict(*pending.pop())
            return OTs

        def proj_residual(w_sb, col_off, n_k, rhs, bp, bbase, xs):
            nxs = []
            for m in range(NDT):
                ps = ps_alt(m)
                for kc in range(n_k):
                    nc.tensor.matmul(
                        ps, w_sb[:, kc, col_off + m * P:col_off + (m + 1) * P],
                        rhs[kc], start=kc == 0, stop=kc == n_k - 1,
                    )
                nx = p_x.tile([P, TOWN], f32, tag="x")
                nc.vector.scalar_tensor_tensor(nx, ps, bp[:, bbase + m:bbase + m + 1],
                                               xs[m], OP.add, OP.add)
                nxs.append(nx)
            return nxs

        def ffn(w1_ap, w2_ap, bp, b1base, b2base, hs, xs):
            gs = []
            for mp in range(F // P // 2):
                w1m = p_w1.tile([P, NDT, 2 * P], bf16, tag="wf1")
                nc.sync.dma_start(
                    out=w1m,
                    in_=w1_ap[:, mp * 2 * P:(mp + 1) * 2 * P].rearrange("(kc p) m -> p kc m", p=P),
                )
                for mi in range(2):
                    m = 2 * mp + mi
                    ps = ps_alt(m)
                    for kc in range(NDT):
                        nc.tensor.matmul(
                            ps, w1m[:, kc, mi * P:(mi + 1) * P], hs[kc],
                            start=kc == 0, stop=kc == NDT - 1,
                        )
                    g = p_g.tile([P, TOWN], bf16, tag="g")
                    nc.scalar.activation(g, ps, AF.Gelu,
                                         bias=bp[:, b1base + m:b1base + m + 1])
                    gs.append(g)
            nxs = []
            for m in range(NDT):
                w2m = p_w2.tile([P, F // P, P], bf16, tag="wf2")
                nc.sync.dma_start(
                    out=w2m,
                    in_=w2_ap[:, m * P:(m + 1) * P].rearrange("(kc p) c -> p kc c", p=P),
                )
                ps2 = ps_alt(m)
                for kc in range(F // P):
                    nc.tensor.matmul(
                        ps2, w2m[:, kc, :], gs[kc],
                        start=kc == 0, stop=kc == F // P - 1,
                    )
                nx = p_x.tile([P, TOWN], f32, tag="x")
                nc.vector.scalar_tensor_tensor(nx, ps2, bp[:, b2base + m:b2base + m + 1],
                                               xs[m], OP.add, OP.add)
                nxs.append(nx)
            return nxs

        def ag_h(hs, uniq):
            """DMA h tiles to a DRAM bounce and AllGather across the pair.
            Returns the gathered [2, NDT, P, TOWN] DRAM tile (index = half)."""
            hbin = p_dram.tile([NDT, P, TOWN], bf16, tag=f"hbin{uniq}")
            for dt in range(NDT):
                nc.sync.dma_start(out=hbin[dt], in_=hs[dt])
            hbout = p_dram.tile([2, NDT, P, TOWN], bf16, tag=f"hbout{uniq}")
            nc.gpsimd.collective_compute(
                "AllGather", OP.bypass, replica_groups=PAIRS,
                ins=[hbin[:].opt()], outs=[hbout[:].opt()],
            )
            return hbout

        def load_h_halves(hbout):
            halves = []
            for hh in range(2):
                tiles = []
                for dt in range(NDT):
                    t = p_h.tile([P, TOWN], bf16, tag="hall")
                    nc.sync.dma_start(out=t, in_=hbout[hh, dt])
                    # keep the HAM clock warm through the AllGather wait: a
                    # dummy matmul pinned on each arriving h tile
                    psW = ps_s.tile([P, T2], f32, tag="pss", name=f"pwh{hh}{dt}")
                    nc.tensor.matmul(psW[:, 0:TOWN], dummy_w, t, start=True, stop=True)
                    tiles.append(t)
                halves.append(tiles)
            return halves

        p_x = ep(tc.tile_pool(name="p_x", bufs=5))
        p_h = ep(tc.tile_pool(name="p_h", bufs=12))
        p_q = ep(tc.tile_pool(name="p_q", bufs=5))
        p_kv = ep(tc.tile_pool(name="p_kv", bufs=8))
        p_v = ep(tc.tile_pool(name="p_v", bufs=9))
        p_ot = ep(tc.tile_pool(name="p_ot", bufs=4))
        p_e = ep(tc.tile_pool(name="p_e", bufs=4))
        p_g = ep(tc.tile_pool(name="p_g", bufs=16))
        p_lnsq = ep(tc.tile_pool(name="p_lnsq", bufs=4))
        p_lnac = ep(tc.tile_pool(name="p_lnac", bufs=2))
        p_bc = ep(tc.tile_pool(name="p_bc", bufs=1))
        p_small = ep(tc.tile_pool(name="p_small", bufs=6))
        p_bias = ep(tc.tile_pool(name="p_bias", bufs=4))
        p_bp = ep(tc.tile_pool(name="p_bp", bufs=2))
        p_eo = ep(tc.tile_pool(name="p_eo", bufs=2))
        p_eoball = ep(tc.tile_pool(name="p_eoball", bufs=4))
        p_w1 = ep(tc.tile_pool(name="p_w1", bufs=2))
        p_w2 = ep(tc.tile_pool(name="p_w2", bufs=2))
        p_w = ep(tc.tile_pool(name="p_w", bufs=3))

        all_kt = list(range(NKT))
        sa_masks = {kt: ("half", causal_sb[kt]) for kt in range(NKT)}
        enc_masks = {kt: ("dup", allow_sb[kt]) for kt in range(NKT)}

        # ================= encoder =================
        xs = []
        for dt in range(NDT):
            x = p_x.tile([P, TOWN], f32, tag="x")
            nc.sync.dma_start(out=x, in_=x0T[dt])
            xs.append(x)

        for l in range(n_enc):
            wq = load_w(p_w, ew_qkv[l][:, 0:D], NDT, D, "w")
            wk = load_w(p_w, ew_qkv[l][:, D:2 * D], NDT, D, "w")
            wv = load_w(p_w, ew_qkv[l][:, 2 * D:3 * D], NDT, D, "w")
            wout = load_w(p_w, ew_out[l], NDT, D, "w")
            bp = p_bp.tile([P, 32], f32, tag="bp")
            nc.sync.dma_start(out=bp, in_=ebp_d[l])

            hs = layer_norm(xs, bf16, p_h, "h")
            hbout = ag_h(hs, f"e{l}")
            Qs = proj_fm(wq, 0, 4, hs, bp, 0, p_q, "q")
            Ks = [p_kv.tile([P, LE], bf16, tag="ksb", name=f"ks{m}") for m in range(NDT)]
            Vs = alloc_vs()
            hhalves = load_h_halves(hbout)
            for hh in range(2):
                kv_project(wk, wv, hhalves[hh], hh, bp, 4, eb_qkv_bf[l], Ks, Vs)
            OTs = attention(Qs, Ks, Vs, all_kt, enc_masks, f"e{l}")
            xs = proj_residual(wout, 0, NDT, OTs, bp, 8, xs)
            hs = layer_norm(xs, bf16, p_h, "h")
            xs = ffn(ew_f1[l], ew_f2[l], bp, 12, 28, hs, xs)

        eof = layer_norm(xs, f32, p_eo, "eof")
        eob = []
        for dt in range(NDT):
            nc.sync.dma_start(out=enc_part[dt], in_=eof[dt])
            t = p_h.tile([P, TOWN], bf16, tag="eob")
            nc.vector.tensor_copy(t, eof[dt])
            eob.append(t)
        ebout = ag_h(eob, "eo")
        eob_all = []
        for dt in range(NDT):
            t = p_eoball.tile([P, LE], bf16, tag="eoball")
            for hh in range(2):
                nc.sync.dma_start(
                    out=t[:, hh * TOWN:(hh + 1) * TOWN], in_=ebout[hh, dt]
                )
            eob_all.append(t)

        # ================= decoder =================
        ys = []
        for dt in range(NDT):
            y = p_x.tile([P, TOWN], f32, tag="x")
            nc.sync.dma_start(out=y, in_=y0T[dt])
            ys.append(y)

        def ca_k_project(l, wkv, bp):
            caK = [p_kv.tile([P, LE], bf16, tag="ksb", name=f"cak{m}") for m in range(NDT)]
            for m in range(NDT):
                ps = ps_mm.tile([P, TOWN], f32, tag="mm")
                for kc in range(NDT):
                    nc.tensor.matmul(
                        ps, wkv[:, kc, m * P:(m + 1) * P],
                        eob_all[kc][:, 0:TOWN], start=kc == 0, stop=kc == NDT - 1,
                    )
                ps2 = ps_s.tile([P, TOWN], f32, tag="pss")
                for kc in range(NDT):
                    nc.tensor.matmul(
                        ps2, wkv[:, kc, m * P:(m + 1) * P],
                        eob_all[kc][:, TOWN:LE], start=kc == 0, stop=kc == NDT - 1,
                    )
                bcol = bp[:, 16 + m:16 + m + 1]
                nc.scalar.activation(caK[m][:, 0:TOWN], ps, AF.Identity, bias=bcol)
                nc.scalar.activation(caK[m][:, TOWN:LE], ps2, AF.Identity, bias=bcol)
            return caK

        def ca_v_project(l):
            wkv = load_w(p_w, dw_caqkv[l][:, 2 * D:3 * D], NDT, D, "w")
            brow512 = p_bias.tile([1, TOWN], bf16, tag="brow512")
            nc.sync.dma_start(
                out=brow512,
                in_=db_caqkv_bf[l][2 * D:3 * D, :].rearrange("a b -> b a"),
            )
            caV = alloc_vs()
            for tt in range(NKT):
                ps = ps_alt(tt)
                for kc in range(NDT):
                    nc.tensor.matmul(
                        ps, eob_all[kc][:, tt * P:(tt + 1) * P],
                        wkv[:, kc, 0:D],
                        start=kc == 0, stop=False,
                    )
                nc.tensor.matmul(ps, ones_row_bf, brow512, start=False, stop=True)
                nc.vector.tensor_copy(
                    caV[tt][:, :, 0:64],
                    ps.rearrange("p (h d) -> p h d", h=H),
                )
            return caV

        for l in range(n_dec):
            wq = load_w(p_w, dw_saqkv[l][:, 0:D], NDT, D, "w")
            wkv_ca = load_w(p_w, dw_caqkv[l][:, D:2 * D], NDT, D, "w")
            wk = load_w(p_w, dw_saqkv[l][:, D:2 * D], NDT, D, "w")
            wv = load_w(p_w, dw_saqkv[l][:, 2 * D:3 * D], NDT, D, "w")
            wout = load_w(p_w, dw_saout[l], NDT, D, "w")
            bp = p_bp.tile([P, 44], f32, tag="bp")
            nc.sync.dma_start(out=bp, in_=dbp_d[l])

            # self-attention (causal)
            hs = layer_norm(ys, bf16, p_h, "h")
            hbout = ag_h(hs, f"d{l}")
            Qs = proj_fm(wq, 0, 4, hs, bp, 0, p_q, "q")
            # independent work to cover the AllGather flight:
            caK = ca_k_project(l, wkv_ca, bp)
            Ks = [p_kv.tile([P, LE], bf16, tag="ksb", name=f"ks{m}") for m in range(NDT)]
            Vs = alloc_vs()
            hhalves = load_h_halves(hbout)
            for hh in range(2):
                kv_project(wk, wv, hhalves[hh], hh, bp, 4, db_saqkv_bf[l], Ks, Vs)
            OTs = attention(Qs, Ks, Vs, all_kt, sa_masks, f"s{l}")
            ys = proj_residual(wout, 0, NDT, OTs, bp, 8, ys)
            caV = ca_v_project(l)  # fills the LN2-chain PE bubble

            # cross-attention (no mask)
            wcaq = load_w(p_w, dw_caqkv[l][:, 0:D], NDT, D, "w")
            wcao = load_w(p_w, dw_caout[l], NDT, D, "w")
            hs = layer_norm(ys, bf16, p_h, "h")
            Qs = proj_fm(wcaq, 0, 4, hs, bp, 12, p_q, "q")
            OTs = attention(Qs, caK, caV, list(range(NKT)), {}, f"c{l}")
            ys = proj_residual(wcao, 0, NDT, OTs, bp, 20, ys)

            # ffn
            hs = layer_norm(ys, bf16, p_h, "h")
            ys = ffn(dw_f1[l], dw_f2[l], bp, 24, 40, hs, ys)

        dof = layer_norm(ys, f32, p_eo, "eof")
        for dt in range(NDT):
            nc.sync.dma_start(out=dec_part[dt], in_=dof[dt])

    nc.compile()
    return nc


def make_in_maps(inputs):
    inp = {k: np.asarray(v) for k, v in inputs.items()}
    f32 = np.float32

    W = {
        "ew_qkv": np.ascontiguousarray(inp["e_qkv_w"].swapaxes(1, 2)).astype(BF16),
        "ew_out": np.ascontiguousarray(inp["e_out_w"].swapaxes(1, 2)).astype(BF16),
        "ew_f1": np.ascontiguousarray(inp["e_ff1_w"].swapaxes(1, 2)).astype(BF16),
        "ew_f2": np.ascontiguousarray(inp["e_ff2_w"].swapaxes(1, 2)).astype(BF16),
        "eb_qkv": inp["e_qkv_b"].astype(f32).reshape(NE, 3 * D, 1),
        "eb_out": inp["e_out_b"].astype(f32).reshape(NE, D, 1),
        "eb_f1": inp["e_ff1_b"].astype(f32).reshape(NE, F, 1),
        "eb_f2": inp["e_ff2_b"].astype(f32).reshape(NE, D, 1),
        "dw_saqkv": np.ascontiguousarray(inp["d_sa_qkv_w"].swapaxes(1, 2)).astype(BF16),
        "db_saqkv": inp["d_sa_qkv_b"].astype(f32).reshape(ND, 3 * D, 1),
        "dw_saout": np.ascontiguousarray(inp["d_sa_out_w"].swapaxes(1, 2)).astype(BF16),
        "db_saout": inp["d_sa_out_b"].astype(f32).reshape(ND, D, 1),
        "dw_caqkv": np.ascontiguousarray(inp["d_ca_qkv_w"].swapaxes(1, 2)).astype(BF16),
        "db_caqkv": inp["d_ca_qkv_b"].astype(f32).reshape(ND, 3 * D, 1),
        "dw_caout": np.ascontiguousarray(inp["d_ca_out_w"].swapaxes(1, 2)).astype(BF16),
        "db_caout": inp["d_ca_out_b"].astype(f32).reshape(ND, D, 1),
        "dw_f1": np.ascontiguousarray(inp["d_ff1_w"].swapaxes(1, 2)).astype(BF16),
        "db_f1": inp["d_ff1_b"].astype(f32).reshape(ND, F, 1),
        "dw_f2": np.ascontiguousarray(inp["d_ff2_w"].swapaxes(1, 2)).astype(BF16),
        "db_f2": inp["d_ff2_b"].astype(f32).reshape(ND, D, 1),
        "eb_qkv_bf": inp["e_qkv_b"].astype(BF16).reshape(NE, 3 * D, 1),
        "ebp": np.stack([
            np.concatenate([
                inp["e_qkv_b"][l][0:2 * D].reshape(8, P).T,
                inp["e_out_b"][l].reshape(4, P).T,
                inp["e_ff1_b"][l].reshape(16, P).T,
                inp["e_ff2_b"][l].reshape(4, P).T,
            ], axis=1).astype(f32) for l in range(NE)]),
        "dbp": np.stack([
            np.concatenate([
                inp["d_sa_qkv_b"][l][0:2 * D].reshape(8, P).T,
                inp["d_sa_out_b"][l].reshape(4, P).T,
                inp["d_ca_qkv_b"][l][0:2 * D].reshape(8, P).T,
                inp["d_ca_out_b"][l].reshape(4, P).T,
                inp["d_ff1_b"][l].reshape(16, P).T,
                inp["d_ff2_b"][l].reshape(4, P).T,
            ], axis=1).astype(f32) for l in range(ND)]),
        "db_saqkv_bf": inp["d_sa_qkv_b"].astype(BF16).reshape(ND, 3 * D, 1),
        "db_caqkv_bf": inp["d_ca_qkv_b"].astype(BF16).reshape(ND, 3 * D, 1),
    }

    in_maps = []
    for c in range(NCORE):
        b, half = c // 2, c % 2
        sl = slice(half * TOWN, (half + 1) * TOWN)
        m = dict(W)
        xT = np.ascontiguousarray(inp["enc_in"][b].astype(f32).T[:, sl])
        m["x0T"] = xT.reshape(NDT, P, TOWN)
        yT = np.ascontiguousarray(inp["dec_in"][b].astype(f32).T[:, sl])
        m["y0T"] = yT.reshape(NDT, P, TOWN)
        xyz = inp["enc_xyz"][b].astype(f32)
        n2 = (xyz * xyz).sum(-1, dtype=f32).astype(f32)
        xq2 = (np.float32(2.0) * xyz[sl]).astype(f32)
        m["xq2"] = np.ascontiguousarray(xq2)
        xkn = np.concatenate([xyz, n2[:, None]], 1).astype(f32)
        m["xrow"] = np.ascontiguousarray(xkn.T)
        bos = np.full((TOWN, 1), 1e30, f32)
        if half == 0:
            bos[0, 0] = NEG
        m["boscol"] = bos
        m["eye"] = np.eye(P, dtype=np.float32)
        # causal tiles vs own queries, absolute key-tile order
        qg = np.arange(half * TOWN, (half + 1) * TOWN)
        kg = np.arange(LE)
        m["causal"] = np.ascontiguousarray(
            (kg[:, None] <= qg[None, :]).astype(BF16)
        ).reshape(NKT, P, TOWN)
        in_maps.append(m)
    return in_maps


def assemble(results):
    enc = np.zeros((B, LE, D), np.float32)
    dec = np.zeros((B, LD, D), np.float32)
    for c, r in enumerate(results):
        b, half = c // 2, c % 2
        sl = slice(half * TOWN, (half + 1) * TOWN)
        enc[b, sl, :] = r["enc_part"].reshape(D, TOWN).T
        dec[b, sl, :] = r["dec_part"].reshape(D, TOWN).T
    return enc, dec


def kernel(**inputs):
    from concourse import bass_utils

    if "nc" not in _CACHE:
        _CACHE["nc"] = build()
    nc = _CACHE["nc"]
    in_maps = make_in_maps(inputs)
    res = bass_utils.run_bass_kernel_spmd(
        nc, in_maps, core_ids=list(range(NCORE))
    )
    return assemble(res.results)
